# revision 26
# baseline (speedup 1.0000x reference)
"""AutoFormer forward pass on 8 Trainium2 NeuronCores (Bass/Tile).

Data-parallel over batch (1 sample per core). All heavy compute in fp16
matmuls on the PE (fp32 PSUM accumulate); vector math in fp32.

Key tricks:
  - Activations kept channel-major [C, L] so every projection/conv is a
    natural PE matmul with pre-transposed weights.
  - AutoCorrelation without FFT: Gram matrix G = q^T k (PE), circular
    diagonal sums extracted with a stride-2049 DMA from a column-duplicated
    DRAM copy of G, reduced with ones-matmuls -> unnormalized mean_corr.
  - Shared top-k delays need the batch mean -> one tiny [1024] fp32
    AllReduce per autocorrelation (3 total).
  - top-20 mask via vector.max + match_replace (no indices anywhere).
  - Delay aggregation as a circulant matmul: softmax weights written to
    DRAM duplicated, re-read as a [128,1024] moving operand with
    per-partition descending offsets (circulant materialization by DMA).
  - Moving average (kernel 25, replicate pad) via fp32 prefix-scan
    (tensor_tensor_scan) + shifted window subtraction + rank-1 edge fixes.
"""

import sys

if "/opt/trn_rl_repo" not in sys.path:
    sys.path.insert(0, "/opt/trn_rl_repo")

import numpy as np

import concourse.bass as bass
import concourse.mybir as mybir
import concourse.tile as tile
from concourse import bacc
from concourse.bass_utils import run_bass_kernel_spmd

F16 = mybir.dt.float16
F32 = mybir.dt.float32
AF = mybir.ActivationFunctionType
ALU = mybir.AluOpType

N_CORES = 8
L = 1024
HID = 512
INF = 64
CF = 2048
KD = 25
TOPK = 20
NT = HID // 128   # 4 channel tiles
CT = CF // 128    # 16 conv-mid channel tiles
NEG = -1e30
SMALLNEG = -1e5

LAST_RESULT = None
_CACHED = None


# --------------------------------------------------------------------------
# program builder
# --------------------------------------------------------------------------

def build_program(n_cores=N_CORES):
    nc = bacc.Bacc("TRN2", target_bir_lowering=False, debug=False,
                   num_devices=n_cores)

    io = {}

    def inp(name, shape, dtype):
        io[name] = nc.dram_tensor(name, list(shape), dtype,
                                  kind="ExternalInput").ap()
        return io[name]

    xt = inp("xt", [INF, L], F32)
    we = inp("we", [INF, HID], F16)
    wf = inp("wf", [HID, INF], F16)
    bemb = inp("bemb", [128, NT], F32)
    bfin = inp("bfin", [128, INF], F32)
    coefL = inp("coefL", [128, 12], F32)
    coefR = inp("coefR", [128, 12], F32)
    onesh = inp("onesh", [128, 128], F16)
    # per-autocorr projection weights [HID, HID] fp16, natural [cin, cout]
    for pfx in ("e", "d1", "d2"):
        for wn in ("wq", "wk", "wv", "wo"):
            inp(f"{pfx}_{wn}", [HID, HID], F16)
    for wn in ("lin1", "lin2", "lin3", "lins"):
        inp(wn, [HID, HID], F16)
    for pfx in ("e", "d"):
        inp(f"{pfx}_w1", [HID, 3 * CF], F16)   # [cin, tap, cout] flat
        inp(f"{pfx}_w2", [CF, 3 * HID], F16)
        inp(f"{pfx}_lng", [128, NT], F32)
        inp(f"{pfx}_lnb", [128, NT], F32)

    out = nc.dram_tensor("out", [512, INF], F32, kind="ExternalOutput").ap()

    with tile.TileContext(nc) as tc:
        _body(tc, io, out, n_cores)
    nc.compile()
    return nc


def _body(tc, io, out, n_cores):
    nc = tc.nc
    import contextlib
    ctx = contextlib.ExitStack()

    wp = ctx.enter_context(tc.tile_pool(name="wp", bufs=1))       # weights
    ap_ = ctx.enter_context(tc.tile_pool(name="ap", bufs=1))      # activations
    fp = ctx.enter_context(tc.tile_pool(name="fp", bufs=1))       # f32 work
    sp = ctx.enter_context(tc.tile_pool(name="sp", bufs=2))       # small
    pm = ctx.enter_context(tc.tile_pool(name="pm", bufs=2, space="PSUM"))
    dp = ctx.enter_context(tc.tile_pool(name="dp", bufs=1, space="DRAM"))
    tc._dbg_pools = [wp, ap_, fp, sp, pm, dp]

    uid = [0]

    def nm(s):
        uid[0] += 1
        return f"{s}{uid[0]}"

    def psum_main(name):
        return pm.tile([128, 1024], F32, tag="pmain", bufs=2, name=nm(name))

    def psum_vec(name):
        return pm.tile([1, 1024], F32, tag="pvec", bufs=2, name=nm(name))

    # ---------------- small constants ----------------
    zc = sp.tile([128, 1], F32, tag="zc", bufs=1, name="zc")
    nc.vector.memset(zc, 0.0)
    epsct = sp.tile([128, 1], F32, tag="epsct", bufs=1, name="epsct")
    nc.vector.memset(epsct, 1e-5)
    # Packed small row-vectors. Compute engines only address partition bases
    # {0,32,64,96}, and 2-input DVE ops need EQUAL input bases — so each
    # phase gets one base (enc=0, dec1=32, dec2=64, layernorms=96) and its
    # vectors live at that base across three big tiles + a tiny scratch.
    svL = sp.tile([128, 1024], F32, tag="svL", bufs=1, name="svL")
    svX = sp.tile([128, 1024], F32, tag="svX", bufs=1, name="svX")
    svY = sp.tile([128, 1024], F32, tag="svY", bufs=1, name="svY")
    svS = sp.tile([128, 32], F32, tag="svS", bufs=1, name="svS")
    svh = sp.tile([128, 2048], F16, tag="svh", bufs=1, name="svh")
    PHASE_BASE = {0: 0, 1: 32, 2: 64}
    ones = sp.tile([128, 128], F16, tag="ones", bufs=1, name="ones")
    nc.sync.dma_start(ones, io["onesh"])
    # coef constants bounce through a DVE copy so downstream DVE ops that
    # also read DMA-produced operands don't exceed per-inst sync-wait slots
    coefLd = sp.tile([128, 12], F32, tag="coefLd", bufs=1, name="coefLd")
    nc.sync.dma_start(coefLd, io["coefL"])
    coefL = sp.tile([128, 12], F32, tag="coefL", bufs=1, name="coefL")
    nc.vector.tensor_copy(coefL, coefLd)
    coefRd = sp.tile([128, 12], F32, tag="coefRd", bufs=1, name="coefRd")
    nc.sync.dma_start(coefRd, io["coefR"])
    coefR = sp.tile([128, 12], F32, tag="coefR", bufs=1, name="coefR")
    nc.vector.tensor_copy(coefR, coefRd)
    bemb = sp.tile([128, NT], F32, tag="bemb", bufs=1, name="bemb")
    nc.sync.dma_start(bemb, io["bemb"])
    bfin = sp.tile([128, INF], F32, tag="bfin", bufs=1, name="bfin")
    nc.sync.dma_start(bfin, io["bfin"])
    lnw = {}
    for pfx in ("e", "d"):
        for wn in ("lng", "lnb"):
            t = sp.tile([128, NT], F32, tag=f"{pfx}{wn}", bufs=1,
                        name=f"{pfx}{wn}")
            nc.sync.dma_start(t, io[f"{pfx}_{wn}"])
            lnw[f"{pfx}{wn}"] = t

    # ---------------- helpers ----------------
    def load_proj_w(name):
        """[HID, ncol] fp16 dram -> NT tiles [128, ncol]."""
        ncol = io[name].shape[1]
        ts = []
        for ci in range(NT):
            t = wp.tile([128, ncol], F16, tag="pw", bufs=8, name=nm(f"{name}_"))
            nc.sync.dma_start(t, io[name][ci * 128:(ci + 1) * 128, :])
            ts.append(t)
        return ts

    def proj_cm(act_h, wts, evict, nci=NT, nco=HID):
        """out[co, l] = sum_ci w[ci, co] act[ci, l]; evict(cc, psum)."""
        for cc in range(nco // 128):
            ps = psum_main("pj")
            for ci in range(nci):
                for h in range(2):
                    nc.tensor.matmul(
                        ps[:, h * 512:(h + 1) * 512],
                        lhsT=wts[ci][:, cc * 128:(cc + 1) * 128],
                        rhs=act_h[ci][:, h * 512:(h + 1) * 512],
                        start=(ci == 0), stop=(ci == nci - 1))
            evict(cc, ps)

    def decomp(a_ap, s_out, t_out=None, cols=L):
        """seasonal/trend decomposition along free dim of one tile.
        a_ap: [P, cols] (fp16 or f32). s_out: seasonal dest (may be None).
        t_out: (dest, scale_into) trend dest fp16 if wanted."""
        cs = fp.tile([128, 1024], F32, tag="cs", bufs=1, name=nm("cs"))
        cs = cs[:a_ap.shape[0], :cols]
        nc.vector.tensor_tensor_scan(
            cs, a_ap, zc[:a_ap.shape[0], :].to_broadcast([a_ap.shape[0], cols]),
            0.0, ALU.add, ALU.add)
        d = fp.tile([128, 1024], F32, tag="dwin", bufs=1, name=nm("dw"))
        d = d[:a_ap.shape[0], :cols]
        n = cols
        nc.vector.tensor_sub(d[:, 13:n - 12], cs[:, 25:n], cs[:, 0:n - 25])
        nc.vector.tensor_copy(d[:, 0:13], cs[:, 12:25])
        nc.vector.scalar_tensor_tensor(
            d[:, n - 12:n], in0=cs[:, n - 25:n - 13], scalar=-1.0,
            in1=cs[:, n - 1:n].to_broadcast([a_ap.shape[0], 12]),
            op0=ALU.mult, op1=ALU.add)
        nc.vector.scalar_tensor_tensor(
            d[:, 0:12], in0=coefL[:a_ap.shape[0], :], scalar=a_ap[:, 0:1],
            in1=d[:, 0:12], op0=ALU.mult, op1=ALU.add)
        nc.vector.scalar_tensor_tensor(
            d[:, n - 12:n], in0=coefR[:a_ap.shape[0], :],
            scalar=a_ap[:, n - 1:n],
            in1=d[:, n - 12:n], op0=ALU.mult, op1=ALU.add)
        if s_out is not None:
            nc.vector.scalar_tensor_tensor(
                s_out, in0=d, scalar=-1.0 / KD, in1=a_ap,
                op0=ALU.mult, op1=ALU.add)
        if t_out is not None:
            nc.vector.tensor_scalar_mul(t_out, d, 1.0 / KD)

    # ================= autocorrelation =================
    def ac_gram(q, k, phase):
        """q,k: NT fp16 tiles [128,1024] (channel-major). Computes local cvec
        and kicks off AllReduce. Returns (lv, gv_dram or None, gdram)."""
        gdram = dp.tile([L, 2 * L], F16, tag="gd", bufs=2, name=nm("gd"))
        for nt in range(8):
            ps = psum_main("gram")
            for ci in range(NT):
                for h in range(2):
                    nc.tensor.matmul(
                        ps[:, h * 512:(h + 1) * 512],
                        lhsT=q[ci][:, nt * 128:(nt + 1) * 128],
                        rhs=k[ci][:, h * 512:(h + 1) * 512],
                        start=(ci == 0), stop=(ci == NT - 1))
            gh = ap_.tile([128, 1024], F16, tag="gh", bufs=2, name=nm("gh"))
            nc.scalar.copy(gh, ps)
            nc.sync.dma_start(gdram[nt * 128:(nt + 1) * 128, 0:L], gh)
            nc.sync.dma_start(gdram[nt * 128:(nt + 1) * 128, L:2 * L], gh)
        pv = psum_vec("cv")
        for nt in range(8):
            dt = ap_.tile([128, 1024], F16, tag="dg", bufs=2, name=nm("dg"))
            src = bass.AP(gdram.tensor, 2049 * 128 * nt + 1,
                          [[2049, 128], [1, 1024]])
            nc.sync.dma_start(dt, src)
            for h in range(2):
                nc.tensor.matmul(pv[:, h * 512:(h + 1) * 512],
                                 lhsT=ones[:, 0:1],
                                 rhs=dt[:, h * 512:(h + 1) * 512],
                                 start=(nt == 0), stop=(nt == 7))
        b = PHASE_BASE[phase]
        lv = svL[b:b + 1, :]
        nc.vector.tensor_copy(lv, pv)
        if n_cores > 1:
            cci = dp.tile([1, 1024], F32, tag="cci", bufs=2, name=nm("cci"))
            cco = dp.tile([1, 1024], F32, tag="cco", bufs=2,
                          addr_space="Shared", name=nm("cco"))
            nc.sync.dma_start(cci, lv)
            nc.gpsimd.collective_compute(
                "AllReduce", ALU.add,
                replica_groups=[list(range(n_cores))],
                ins=[cci.opt()], outs=[cco.opt()])
            gv = svX[b:b + 1, :]
            nc.sync.dma_start(gv, cco)
        else:
            gv = lv
        return lv, gv

    def ac_weights(lv, gv, phase):
        """top-20 mask from gv, softmax of lv/HID over mask -> circulant
        weights written duplicated to DRAM. Returns wd dram tile.
        All vectors at this phase's partition base; X/Y buffers rotate."""
        b = PHASE_BASE[phase]
        S = svS[b:b + 1, :]
        X = svX[b:b + 1, :]
        Y = svY[b:b + 1, :]
        m8, m8b, m8c = S[:, 0:8], S[:, 8:16], S[:, 16:24]
        mx, mxn, sm, rc = (S[:, 24:25], S[:, 25:26], S[:, 26:27], S[:, 27:28])
        nc.vector.max(m8, gv)
        nc.vector.match_replace(Y, in_to_replace=m8, in_values=gv,
                                imm_value=NEG)                    # cur1 = Y
        nc.vector.max(m8b, Y)
        nc.vector.match_replace(X, in_to_replace=m8b, in_values=Y,
                                imm_value=NEG)                    # cur2 = X
        nc.vector.max(m8c, X)
        nc.vector.memset(m8c[:, TOPK - 16:8], NEG)
        nc.vector.match_replace(Y, in_to_replace=m8c, in_values=X,
                                imm_value=NEG)                    # cur3 = Y
        # masked logits: msk*(lv - SMALLNEG) + SMALLNEG
        nc.vector.tensor_scalar(X, Y, float(NEG), None, op0=ALU.is_le)  # msk
        nc.vector.tensor_scalar_add(Y, lv, -float(SMALLNEG))           # ml0
        nc.vector.tensor_mul(X, X, Y)
        nc.vector.tensor_scalar_add(Y, X, float(SMALLNEG))             # ml
        nc.vector.reduce_max(mx, Y, axis=mybir.AxisListType.X)
        nc.vector.tensor_scalar_mul(mxn, mx, -1.0 / HID)
        nc.scalar.activation(X, Y, AF.Exp, bias=mxn, scale=1.0 / HID)  # ex
        nc.vector.reduce_sum(sm, X, axis=mybir.AxisListType.X)
        nc.vector.reciprocal(rc, sm)
        nc.vector.tensor_scalar_mul(Y, X, rc)                          # w
        # circulant source buffer: B[j] = W[(j-1) mod 1024], length 129*1024.
        # R_mt[p, l] = W[(l - (128*mt+p) + 1023) mod 1024] = B[(1024-128*mt)
        # + 1023*p + l] — all-positive strides for the DMA.
        wrot = svh[32 * (phase + 1):32 * (phase + 1) + 1, 0:1024]
        nc.scalar.copy(wrot[:, 0:1], Y[:, 1023:1024])
        nc.scalar.copy(wrot[:, 1:1024], Y[:, 0:1023])
        wd = dp.tile([129 * 1024], F16, tag="wd", bufs=2, name=nm("wd"))
        src = wrot.unsqueeze(1).to_broadcast([1, 129, 1024])
        nc.sync.dma_start(wd, src)
        return wd

    def ac_apply(wd, v, wo_ts, resid_h, a_out_tiles):
        """circulant matmul with v (seq-major tiles), then wo projection with
        residual add. a_out_tiles: list of 4 dest tiles (fp16) for
        wo_out + resid."""
        agg = []
        for cp in range(0, NT, 2):
            pss = [psum_main("circ"), psum_main("circ")]
            for mt in range(8):
                rt = ap_.tile([128, 1024], F16, tag="rt", bufs=2, name=nm("rt"))
                src = bass.AP(wd.tensor, 1024 - 128 * mt,
                              [[1023, 128], [1, 1024]])
                nc.sync.dma_start(rt, src)
                for j, ps in enumerate(pss):
                    cc = cp + j
                    for h in range(2):
                        nc.tensor.matmul(
                            ps[:, h * 512:(h + 1) * 512],
                            lhsT=v[mt][:, cc * 128:(cc + 1) * 128],
                            rhs=rt[:, h * 512:(h + 1) * 512],
                            start=(mt == 0), stop=(mt == 7))
            for j, ps in enumerate(pss):
                ag = ap_.tile([128, 1024], F16, tag=f"th{cp + j}", bufs=1,
                              name=nm("agg"))
                nc.scalar.copy(ag, ps)
                agg.append(ag)

        def evict(cc, ps):
            nc.vector.tensor_add(a_out_tiles[cc], ps, resid_h[cc])
        proj_cm(agg, wo_ts, evict)

    def proj_sm(act_h, wts, tag):
        """v seq-major: out[l, co] = sum_ci act[ci, l-chunk]^T w[ci, co].
        Returns 8 tiles [128, 512] fp16."""
        vts = []
        for lt in range(8):
            ps = pm.tile([128, 512], F32, tag="pmain", bufs=2, name=nm("pv"))
            for ci in range(NT):
                nc.tensor.matmul(ps,
                                 lhsT=act_h[ci][:, lt * 128:(lt + 1) * 128],
                                 rhs=wts[ci],
                                 start=(ci == 0), stop=(ci == NT - 1))
            vt = ap_.tile([128, 512], F16, tag=f"v{lt}", bufs=1, name=nm("v"))
            nc.scalar.copy(vt, ps)
            vts.append(vt)
        return vts

    # ================= conv block =================
    def leaky_evict(dst_ap, ps_ap, n):
        """dst = leaky_relu(ps) = max(ps, 0.01*ps), via fp16 tmp."""
        tmp = ap_.tile([128, 512], F16, tag="lkt", bufs=3, name=nm("lkt"))
        nc.scalar.copy(tmp[:, 0:n], ps_ap)
        nc.vector.scalar_tensor_tensor(
            dst_ap, in0=tmp[:, 0:n], scalar=0.01, in1=tmp[:, 0:n],
            op0=ALU.mult, op1=ALU.max)

    def conv_block(sptiles, w1name, w2name, cf_tag):
        """sptiles: 4 fp16 tiles [128, 1026] (replicate-padded seasonal).
        Returns 4 fp16 tiles [128, 1024]: leaky(conv2(leaky(conv1(s)))) + s."""
        w1, w2 = io[w1name], io[w2name]
        cf = [ap_.tile([128, 1024], F16, tag=f"{cf_tag}{i}", bufs=1,
                       name=nm("cf")) for i in range(NT)]
        for lh in range(2):
            c1h = []
            for co in range(CT):
                w1t = []
                for ci in range(NT):
                    t = wp.tile([128, 384], F16, tag=f"w1_{ci}", bufs=2,
                                name=nm("w1t"))
                    src = bass.AP(w1.tensor,
                                  ci * 128 * (3 * CF) + co * 128,
                                  [[3 * CF, 128], [CF, 3], [1, 128]])
                    nc.sync.dma_start(t.rearrange("p (t c) -> p t c", t=3), src)
                    w1t.append(t)
                # 513 true outputs j in [lh*511, lh*511 + 513), computed as
                # 257 cols in psum bank 0 + 256 cols at bank-1 start (col 512)
                # so no matmul crosses a psum bank boundary
                ps = pm.tile([128, 1024], F32, tag="pmain", bufs=2,
                             name=nm("pc1"))
                j0 = 511 * lh
                for ci in range(NT):
                    for tp in range(3):
                        first = (ci == 0 and tp == 0)
                        last = (ci == NT - 1 and tp == 2)
                        for (o0, j1, n) in ((0, 0, 257), (512, 257, 256)):
                            nc.tensor.matmul(
                                ps[:, o0:o0 + n],
                                lhsT=w1t[ci][:, tp * 128:(tp + 1) * 128],
                                rhs=sptiles[ci][:, j0 + j1 + tp:
                                                j0 + j1 + tp + n],
                                start=first, stop=last)
                ch = ap_.tile([128, 514], F16, tag=f"c1_{co}", bufs=1,
                              name=nm("c1h"))
                # tile col t holds output j = lh*511 + t - (1-lh)
                if lh == 0:
                    leaky_evict(ch[:, 1:258], ps[:, 0:257], 257)
                    leaky_evict(ch[:, 258:514], ps[:, 512:768], 256)
                    nc.vector.tensor_copy(ch[:, 0:1], ch[:, 1:2])
                else:
                    leaky_evict(ch[:, 0:257], ps[:, 0:257], 257)
                    leaky_evict(ch[:, 257:513], ps[:, 512:768], 256)
                    nc.vector.tensor_copy(ch[:, 513:514], ch[:, 512:513])
                c1h.append(ch)
            # conv2 for this half: out cols [lh*512, lh*512+512)
            for co in range(NT):
                w2t = []
                for ci in range(CT):
                    t = wp.tile([128, 384], F16, tag=f"w2_{ci}", bufs=1,
                                name=nm("w2t"))
                    src = bass.AP(w2.tensor,
                                  ci * 128 * (3 * HID) + co * 128,
                                  [[3 * HID, 128], [HID, 3], [1, 128]])
                    nc.sync.dma_start(t.rearrange("p (t c) -> p t c", t=3), src)
                    w2t.append(t)
                ps = pm.tile([128, 512], F32, tag="pmain", bufs=2,
                             name=nm("pc2"))
                # c2 out col l = lh*512 + t needs c1[l + tp - 1], which lives
                # at c1h col t + tp for both halves
                for ci in range(CT):
                    for tp in range(3):
                        nc.tensor.matmul(
                            ps,
                            lhsT=w2t[ci][:, tp * 128:(tp + 1) * 128],
                            rhs=c1h[ci][:, tp:tp + 512],
                            start=(ci == 0 and tp == 0),
                            stop=(ci == CT - 1 and tp == 2))
                tmp = ap_.tile([128, 512], F16, tag="c2t", bufs=2,
                               name=nm("c2t"))
                leaky_evict(tmp, ps, 512)
                nc.vector.tensor_add(
                    cf[co][:, lh * 512:(lh + 1) * 512], tmp,
                    sptiles[co][:, lh * 512 + 1:lh * 512 + 513])
        return cf

    # ================= layer norm =================
    def layer_norm(x_h, g_ap, b_ap, out_tiles, lnphase):
        """x_h: 4 fp16 tiles [128,1024] channel-major. out: 4 fp16 tiles."""
        psx = psum_vec("lnx")
        psq = psum_vec("lnq")
        for ci in range(NT):
            sq = ap_.tile([128, 1024], F16, tag="sqh", bufs=2, name=nm("sq"))
            nc.scalar.square(sq, x_h[ci])
            for h in range(2):
                nc.tensor.matmul(psx[:, h * 512:(h + 1) * 512],
                                 lhsT=ones[:, 0:1],
                                 rhs=x_h[ci][:, h * 512:(h + 1) * 512],
                                 start=(ci == 0), stop=(ci == NT - 1))
                nc.tensor.matmul(psq[:, h * 512:(h + 1) * 512],
                                 lhsT=ones[:, 0:1],
                                 rhs=sq[:, h * 512:(h + 1) * 512],
                                 start=(ci == 0), stop=(ci == NT - 1))
        mean = svL[96:97, :]
        nc.vector.tensor_scalar_mul(mean, psx, 1.0 / HID)
        var = svX[96:97, :]
        # var = psq/HID - mean^2
        nc.vector.tensor_mul(var, mean, mean)
        nc.vector.scalar_tensor_tensor(var, in0=psq, scalar=1.0 / HID,
                                       in1=var, op0=ALU.mult, op1=ALU.subtract)
        std = svY[96:97, :]
        nc.scalar.activation(std, var, AF.Sqrt, bias=epsct[96:97, :])
        rstd = svX[96:97, :]   # var dead
        nc.vector.reciprocal(rstd, std)
        mh = svh[0:1, 0:1024]
        nc.vector.tensor_copy(mh, mean)
        rh = svh[0:1, 1024:2048]
        nc.vector.tensor_copy(rh, rstd)
        # broadcast via rank-1 matmul
        psb_m = psum_main("bcm")
        psb_r = psum_main("bcr")
        for h in range(2):
            nc.tensor.matmul(psb_m[:, h * 512:(h + 1) * 512],
                             lhsT=ones[0:1, 0:128],
                             rhs=mh[:, h * 512:(h + 1) * 512],
                             start=True, stop=True)
            nc.tensor.matmul(psb_r[:, h * 512:(h + 1) * 512],
                             lhsT=ones[0:1, 0:128],
                             rhs=rh[:, h * 512:(h + 1) * 512],
                             start=True, stop=True)
        for ci in range(NT):
            t1 = ap_.tile([128, 1024], F16, tag="lnt", bufs=2, name=nm("lnt"))
            nc.vector.tensor_sub(t1, x_h[ci], psb_m)
            nc.vector.tensor_mul(t1, t1, psb_r)
            nc.vector.tensor_scalar(out_tiles[ci], t1,
                                    g_ap[:, ci:ci + 1], b_ap[:, ci:ci + 1],
                                    op0=ALU.mult, op1=ALU.add)

    # ======================================================================
    # phase A — input prep + embeddings
    # ======================================================================
    with nc.named_scope("prep"):
        xtd = fp.tile([INF, 1024], F32, tag="cs", bufs=1, name="xtd")
        nc.sync.dma_start(xtd, io["xt"])
        # DVE bounce: gives scan/stt consumers a same-engine producer
        xtf = fp.tile([INF, 1024], F32, tag="af", bufs=2, name="xtf")
        nc.vector.tensor_copy(xtf, xtd)
        xth = ap_.tile([INF, 1024], F16, tag="lnt", bufs=2, name="xth")
        nc.scalar.copy(xth, xtf)

    # initial decomposition: need the window-sum tensor D directly (both
    # seasonal and trend halves are consumed), so inline it here
    with nc.named_scope("initdecomp"):
        dx = fp.tile([INF, 1024], F32, tag="dwin", bufs=1, name="dx")
        cs0 = fp.tile([128, 1024], F32, tag="cs", bufs=1, name="cs0")
        cs0 = cs0[:INF, :]
        nc.vector.tensor_tensor_scan(
            cs0, xtf, zc[:INF, :].to_broadcast([INF, 1024]),
            0.0, ALU.add, ALU.add)
        d0 = dx
        nc.vector.tensor_sub(d0[:, 13:1012], cs0[:, 25:1024], cs0[:, 0:999])
        nc.vector.tensor_copy(d0[:, 0:13], cs0[:, 12:25])
        nc.vector.scalar_tensor_tensor(
            d0[:, 1012:1024], in0=cs0[:, 999:1011], scalar=-1.0,
            in1=cs0[:, 1023:1024].to_broadcast([INF, 12]),
            op0=ALU.mult, op1=ALU.add)
        nc.vector.scalar_tensor_tensor(
            d0[:, 0:12], in0=coefL[:INF, :], scalar=xtf[:, 0:1],
            in1=d0[:, 0:12], op0=ALU.mult, op1=ALU.add)
        nc.vector.scalar_tensor_tensor(
            d0[:, 1012:1024], in0=coefR[:INF, :], scalar=xtf[:, 1023:1024],
            in1=d0[:, 1012:1024], op0=ALU.mult, op1=ALU.add)

        # seasonal_init: cols 0:512 = (x - D/25)[:, 512:], cols 512: = 0
        sih = ap_.tile([INF, 1024], F16, tag="sqh", bufs=2, name="sih")
        nc.vector.scalar_tensor_tensor(
            sih[:, 0:512], in0=d0[:, 512:1024], scalar=-1.0 / KD,
            in1=xtf[:, 512:1024], op0=ALU.mult, op1=ALU.add)
        nc.vector.memset(sih[:, 512:1024], 0.0)
        # trend_init: cols 0:512 = D[:, 512:]/25, cols 512: = mean(x)
        tih = ap_.tile([INF, 1024], F16, tag="sqh", bufs=2, name="tih")
        nc.vector.tensor_scalar_mul(tih[:, 0:512], d0[:, 512:1024], 1.0 / KD)
        mnx = sp.tile([INF, 1], F32, tag="mnx", bufs=1, name="mnx")
        nc.vector.reduce_sum(mnx, xtf, axis=mybir.AxisListType.X)
        nc.vector.tensor_scalar_mul(mnx, mnx, 1.0 / L)
        nc.vector.tensor_scalar(
            tih[:, 512:1024],
            zc[:INF, :].to_broadcast([INF, 512]), mnx, None,
            op0=ALU.add)

    with nc.named_scope("embed"):
        wemb = [wp.tile([INF, HID], F16, tag="pw", bufs=8, name="wemb")]
        nc.sync.dma_start(wemb[0], io["we"])

        def emb(src_h, dst_tag, f32_dst=False, bufs=1):
            outs = []
            for cc in range(NT):
                ps = psum_main("emb")
                for h in range(2):
                    nc.tensor.matmul(
                        ps[:, h * 512:(h + 1) * 512],
                        lhsT=wemb[0][:, cc * 128:(cc + 1) * 128],
                        rhs=src_h[:, h * 512:(h + 1) * 512],
                        start=True, stop=True)
                if f32_dst:
                    o = fp.tile([128, 1024], F32, tag=f"{dst_tag}{cc}",
                                bufs=bufs, name=nm(dst_tag))
                    nc.vector.tensor_scalar_add(o, ps, bemb[:, cc:cc + 1])
                else:
                    o = ap_.tile([128, 1024], F16, tag=f"{dst_tag}{cc}",
                                 bufs=bufs, name=nm(dst_tag))
                    nc.scalar.activation(o, ps, AF.Identity,
                                         bias=bemb[:, cc:cc + 1])
                outs.append(o)
            return outs

        xe = emb(xth, "xe")
        se = emb(sih, "se")
        trend = emb(tih, "tr", f32_dst=True)  # running trend accumulator

    # ======================================================================
    # encoder autocorrelation (+ dec-ac1 gram interleaved to hide latency)
    # ======================================================================
    with nc.named_scope("enc_ac_gram"):
        wq = load_proj_w("e_wq")
        wk = load_proj_w("e_wk")
        q = []
        k = []
        for cc in range(NT):
            qt = ap_.tile([128, 1024], F16, tag=f"q{cc}", bufs=1, name=nm("q"))
            kt = ap_.tile([128, 1024], F16, tag=f"k{cc}", bufs=1, name=nm("k"))
            q.append(qt)
            k.append(kt)
        proj_cm(xe, wq, lambda cc, ps: nc.scalar.copy(q[cc], ps))
        proj_cm(xe, wk, lambda cc, ps: nc.scalar.copy(k[cc], ps))
        lv_e, gv_e = ac_gram(q, k, 0)

    with nc.named_scope("dec1_ac_gram"):
        wq1 = load_proj_w("d1_wq")
        wk1 = load_proj_w("d1_wk")
        q1 = []
        k1 = []
        for cc in range(NT):
            qt = ap_.tile([128, 1024], F16, tag=f"q{cc}", bufs=1, name=nm("q1"))
            kt = ap_.tile([128, 1024], F16, tag=f"k{cc}", bufs=1, name=nm("k1"))
            q1.append(qt)
            k1.append(kt)
        proj_cm(se, wq1, lambda cc, ps: nc.scalar.copy(q1[cc], ps))
        proj_cm(se, wk1, lambda cc, ps: nc.scalar.copy(k1[cc], ps))
        lv_1, gv_1 = ac_gram(q1, k1, 1)

    with nc.named_scope("enc_ac_apply"):
        wv_ = load_proj_w("e_wv")
        v = proj_sm(xe, wv_, "v")
        wd_e = ac_weights(lv_e, gv_e, 0)
        wo_ = load_proj_w("e_wo")
        a_enc = [fp.tile([128, 1024], F32, tag="af", bufs=2, name=nm("ae"))
                 for _ in range(NT)]
        ac_apply(wd_e, v, wo_, xe, a_enc)

    # ======================================================================
    # encoder decomp 1 -> s1 (padded), convs, decomp 2, LN
    # ======================================================================
    with nc.named_scope("enc_decomp1"):
        s1p = [ap_.tile([128, 1026], F16, tag=f"sp{i}", bufs=1, name=nm("s1p"))
               for i in range(NT)]
        for cc in range(NT):
            decomp(a_enc[cc], s1p[cc][:, 1:1025])
            nc.vector.tensor_copy(s1p[cc][:, 0:1], s1p[cc][:, 1:2])
            nc.vector.tensor_copy(s1p[cc][:, 1025:1026], s1p[cc][:, 1024:1025])

    with nc.named_scope("enc_convs"):
        cf_e = conv_block(s1p, "e_w1", "e_w2", "cf")

    with nc.named_scope("enc_ln"):
        sf = [ap_.tile([128, 1024], F16, tag=f"sf{i}", bufs=1, name=nm("sf"))
              for i in range(NT)]
        for cc in range(NT):
            decomp(cf_e[cc], sf[cc])
        enc_out = [ap_.tile([128, 1024], F16, tag=f"eo{i}", bufs=1,
                            name=nm("eo")) for i in range(NT)]
        layer_norm(sf, lnw["elng"], lnw["elnb"], enc_out, 0)

    # ======================================================================
    # decoder ac1 apply -> a1 -> decomp -> s1d, t1 -> lin1 -> trend
    # ======================================================================
    with nc.named_scope("dec1_apply"):
        wv1 = load_proj_w("d1_wv")
        v1 = proj_sm(se, wv1, "v")
        wd_1 = ac_weights(lv_1, gv_1, 1)
        wo1 = load_proj_w("d1_wo")
        a1 = [fp.tile([128, 1024], F32, tag="af", bufs=2, name=nm("a1"))
              for _ in range(NT)]
        ac_apply(wd_1, v1, wo1, se, a1)

    with nc.named_scope("dec1_decomp"):
        s1d = [ap_.tile([128, 1024], F16, tag=f"s1d{i}", bufs=1, name=nm("s1d"))
               for i in range(NT)]
        th = [ap_.tile([128, 1024], F16, tag=f"th{i}", bufs=1, name=nm("th"))
              for i in range(NT)]
        for cc in range(NT):
            decomp(a1[cc], s1d[cc], t_out=th[cc])
        wl1 = load_proj_w("lin1")

        def ev_t1(cc, ps):
            nc.vector.tensor_add(trend[cc], trend[cc], ps)
        proj_cm(th, wl1, ev_t1)

    # ======================================================================
    # decoder ac2: q from s1d, k/v from enc_out
    # ======================================================================
    with nc.named_scope("dec2_ac"):
        wq2 = load_proj_w("d2_wq")
        wk2 = load_proj_w("d2_wk")
        q2 = []
        k2 = []
        for cc in range(NT):
            qt = ap_.tile([128, 1024], F16, tag=f"q{cc}", bufs=1, name=nm("q2"))
            kt = ap_.tile([128, 1024], F16, tag=f"k{cc}", bufs=1, name=nm("k2"))
            q2.append(qt)
            k2.append(kt)
        proj_cm(s1d, wq2, lambda cc, ps: nc.scalar.copy(q2[cc], ps))
        proj_cm(enc_out, wk2, lambda cc, ps: nc.scalar.copy(k2[cc], ps))
        lv_2, gv_2 = ac_gram(q2, k2, 2)
        wv2 = load_proj_w("d2_wv")
        v2 = proj_sm(enc_out, wv2, "v")
        wd_2 = ac_weights(lv_2, gv_2, 2)
        wo2 = load_proj_w("d2_wo")
        a2 = [fp.tile([128, 1024], F32, tag="af", bufs=2, name=nm("a2"))
              for _ in range(NT)]
        ac_apply(wd_2, v2, wo2, s1d, a2)

    with nc.named_scope("dec2_decomp"):
        s2p = [ap_.tile([128, 1026], F16, tag=f"sp{i}", bufs=1, name=nm("s2p"))
               for i in range(NT)]
        th2 = [ap_.tile([128, 1024], F16, tag=f"th{i}", bufs=1, name=nm("th2"))
               for i in range(NT)]
        for cc in range(NT):
            decomp(a2[cc], s2p[cc][:, 1:1025], t_out=th2[cc])
            nc.vector.tensor_copy(s2p[cc][:, 0:1], s2p[cc][:, 1:2])
            nc.vector.tensor_copy(s2p[cc][:, 1025:1026], s2p[cc][:, 1024:1025])
        wl2 = load_proj_w("lin2")

        def ev_t2(cc, ps):
            nc.vector.tensor_add(trend[cc], trend[cc], ps)
        proj_cm(th2, wl2, ev_t2)

    # ======================================================================
    # decoder convs, decomp 3, LN, season + trend, final projection
    # ======================================================================
    with nc.named_scope("dec_convs"):
        cf_d = conv_block(s2p, "d_w1", "d_w2", "cf")

    with nc.named_scope("dec_final"):
        s3 = [ap_.tile([128, 1024], F16, tag=f"sf{i}", bufs=1, name=nm("s3"))
              for i in range(NT)]
        th3 = [ap_.tile([128, 1024], F16, tag=f"th{i}", bufs=1, name=nm("th3"))
               for i in range(NT)]
        for cc in range(NT):
            decomp(cf_d[cc], s3[cc], t_out=th3[cc])
        wl3 = load_proj_w("lin3")

        def ev_t3(cc, ps):
            nc.vector.tensor_add(trend[cc], trend[cc], ps)
        proj_cm(th3, wl3, ev_t3)

        sea = [ap_.tile([128, 1024], F16, tag=f"xe{i}", bufs=1, name=nm("sea"))
               for i in range(NT)]
        layer_norm(s3, lnw["dlng"], lnw["dlnb"], sea, 1)
        wls = load_proj_w("lins")
        fin = [ap_.tile([128, 1024], F16, tag=f"q{i}", bufs=1,
                        name=nm("fin")) for i in range(NT)]

        def ev_sea(cc, ps):
            nc.vector.tensor_add(fin[cc], ps, trend[cc])
        proj_cm(sea, wls, ev_sea)

        # final: out[l, co] for l in [512, 1024)
        wfin = [wp.tile([128, INF], F16, tag="pw", bufs=8, name=nm("wfin"))
                for _ in range(NT)]
        for ci in range(NT):
            nc.sync.dma_start(wfin[ci], io["wf"][ci * 128:(ci + 1) * 128, :])
        for lt in range(4, 8):
            ps = pm.tile([128, INF], F32, tag="pmain", bufs=2, name=nm("pf"))
            for ci in range(NT):
                nc.tensor.matmul(ps,
                                 lhsT=fin[ci][:, lt * 128:(lt + 1) * 128],
                                 rhs=wfin[ci],
                                 start=(ci == 0), stop=(ci == NT - 1))
            of = sp.tile([128, INF], F32, tag="of", bufs=2, name=nm("of"))
            nc.vector.tensor_add(of, ps, bfin)
            nc.sync.dma_start(
                out[(lt - 4) * 128:(lt - 3) * 128, :], of)

    ctx.close()


# --------------------------------------------------------------------------
# host driver
# --------------------------------------------------------------------------

def _prep_inputs(x, params):
    """Returns list of per-core in_maps."""
    g = lambda *ks: np.asarray(_dig(params, ks))
    shared = {}
    shared["we"] = g("w_emb").astype(np.float16)
    shared["wf"] = g("w_final").astype(np.float16)
    shared["bemb"] = np.ascontiguousarray(
        g("b_emb").reshape(NT, 128).T.astype(np.float32))
    bf = g("b_final").astype(np.float32)
    shared["bfin"] = np.ascontiguousarray(
        np.broadcast_to(bf[None, :], (128, INF)).copy())
    shared["coefL"] = np.ascontiguousarray(np.broadcast_to(
        np.arange(12, 0, -1, dtype=np.float32)[None, :], (128, 12)).copy())
    shared["coefR"] = np.ascontiguousarray(np.broadcast_to(
        np.arange(1, 13, dtype=np.float32)[None, :], (128, 12)).copy())
    shared["onesh"] = np.ones((128, 128), np.float16)
    acmap = {"e": ("enc", "ac"), "d1": ("dec", "ac1"), "d2": ("dec", "ac2")}
    for pfx, ks in acmap.items():
        for wn in ("wq", "wk", "wv", "wo"):
            shared[f"{pfx}_{wn}"] = g(*ks, wn).astype(np.float16)
    for i, wn in enumerate(("lin1", "lin2", "lin3")):
        shared[wn] = g("dec", wn).astype(np.float16)
    shared["lins"] = g("dec", "lin_season").astype(np.float16)
    for pfx, side in (("e", "enc"), ("d", "dec")):
        w1 = g(side, "conv1")  # [CF, HID, 3]
        shared[f"{pfx}_w1"] = np.ascontiguousarray(
            w1.transpose(1, 2, 0).reshape(HID, 3 * CF)).astype(np.float16)
        w2 = g(side, "conv2")  # [HID, CF, 3]
        shared[f"{pfx}_w2"] = np.ascontiguousarray(
            w2.transpose(1, 2, 0).reshape(CF, 3 * HID)).astype(np.float16)
        shared[f"{pfx}_lng"] = np.ascontiguousarray(
            g(side, "ln_g").reshape(NT, 128).T.astype(np.float32))
        shared[f"{pfx}_lnb"] = np.ascontiguousarray(
            g(side, "ln_b").reshape(NT, 128).T.astype(np.float32))

    in_maps = []
    for c in range(N_CORES):
        m = dict(shared)
        m["xt"] = np.ascontiguousarray(np.asarray(x[c]).T.astype(np.float32))
        in_maps.append(m)
    return in_maps


def _dig(d, ks):
    for k in ks:
        d = d[k]
    return d


def kernel(x, params):
    global LAST_RESULT, _CACHED
    import os
    try:
        import antenv.axon_hooks  # noqa: F401
    except ImportError:
        # tracing under axon needs this hook; without it a stray BASS_TRACE
        # in the environment would crash the run
        os.environ["BASS_NEVER_TRACE"] = "1"
    x = np.asarray(x)
    if _CACHED is None:
        _CACHED = build_program(N_CORES)
    nc = _CACHED
    in_maps = _prep_inputs(x, params)
    res = run_bass_kernel_spmd(nc, in_maps, core_ids=list(range(N_CORES)))
    LAST_RESULT = res
    outs = np.stack([r["out"] for r in res.results], axis=0)
    return outs.astype(np.float32)


if __name__ == "__main__":
    nc = build_program(1)
    print("program built OK")


# revision 29
# speedup vs baseline: 1.0274x; 1.0274x over previous
"""AutoFormer forward pass on 8 Trainium2 NeuronCores (Bass/Tile).

Data-parallel over batch (1 sample per core). All heavy compute in fp16
matmuls on the PE (fp32 PSUM accumulate); vector math in fp32.

Key tricks:
  - Activations kept channel-major [C, L] so every projection/conv is a
    natural PE matmul with pre-transposed weights.
  - AutoCorrelation without FFT: Gram matrix G = q^T k (PE), circular
    diagonal sums extracted with a stride-2049 DMA from a column-duplicated
    DRAM copy of G, reduced with ones-matmuls -> unnormalized mean_corr.
  - Shared top-k delays need the batch mean -> one tiny [1024] fp32
    AllReduce per autocorrelation (3 total).
  - top-20 mask via vector.max + match_replace (no indices anywhere).
  - Delay aggregation as a circulant matmul: softmax weights written to
    DRAM duplicated, re-read as a [128,1024] moving operand with
    per-partition descending offsets (circulant materialization by DMA).
  - Moving average (kernel 25, replicate pad) via fp32 prefix-scan
    (tensor_tensor_scan) + shifted window subtraction + rank-1 edge fixes.
"""

import sys

if "/opt/trn_rl_repo" not in sys.path:
    sys.path.insert(0, "/opt/trn_rl_repo")

import numpy as np

import concourse.bass as bass
import concourse.mybir as mybir
import concourse.tile as tile
from concourse import bacc
from concourse.bass_utils import run_bass_kernel_spmd

F16 = mybir.dt.float16
F32 = mybir.dt.float32
AF = mybir.ActivationFunctionType
ALU = mybir.AluOpType

N_CORES = 8
L = 1024
HID = 512
INF = 64
CF = 2048
KD = 25
TOPK = 20
NT = HID // 128   # 4 channel tiles
CT = CF // 128    # 16 conv-mid channel tiles
NEG = -1e30
SMALLNEG = -1e5

LAST_RESULT = None
_CACHED = None


# --------------------------------------------------------------------------
# program builder
# --------------------------------------------------------------------------

def build_program(n_cores=N_CORES):
    nc = bacc.Bacc("TRN2", target_bir_lowering=False, debug=False,
                   num_devices=n_cores)

    io = {}

    def inp(name, shape, dtype):
        io[name] = nc.dram_tensor(name, list(shape), dtype,
                                  kind="ExternalInput").ap()
        return io[name]

    xt = inp("xt", [INF, L], F32)
    we = inp("we", [INF, HID], F16)
    wf = inp("wf", [HID, INF], F16)
    bemb = inp("bemb", [128, NT], F32)
    bfin = inp("bfin", [128, INF], F32)
    coefL = inp("coefL", [128, 12], F32)
    coefR = inp("coefR", [128, 12], F32)
    onesh = inp("onesh", [128, 128], F16)
    # per-autocorr projection weights [HID, HID] fp16, natural [cin, cout]
    for pfx in ("e", "d1", "d2"):
        for wn in ("wq", "wk", "wv", "wo"):
            inp(f"{pfx}_{wn}", [HID, HID], F16)
    for wn in ("lin1", "lin2", "lin3", "lins"):
        inp(wn, [HID, HID], F16)
    for pfx in ("e", "d"):
        # conv weights pre-gathered per 128-wide output chunk so each
        # [128,384] SBUF tile is one DMA of contiguous 768B rows
        inp(f"{pfx}_w1", [CT * HID, 384], F16)   # [co_chunk, cin, tap*128]
        inp(f"{pfx}_w2", [NT * CF, 384], F16)
        inp(f"{pfx}_lng", [128, NT], F32)
        inp(f"{pfx}_lnb", [128, NT], F32)

    out = nc.dram_tensor("out", [512, INF], F32, kind="ExternalOutput").ap()

    with tile.TileContext(nc) as tc:
        _body(tc, io, out, n_cores)
    nc.compile()
    return nc


def _body(tc, io, out, n_cores):
    nc = tc.nc
    import contextlib
    ctx = contextlib.ExitStack()

    wp = ctx.enter_context(tc.tile_pool(name="wp", bufs=1))       # weights
    ap_ = ctx.enter_context(tc.tile_pool(name="ap", bufs=1))      # activations
    fp = ctx.enter_context(tc.tile_pool(name="fp", bufs=1))       # f32 work
    sp = ctx.enter_context(tc.tile_pool(name="sp", bufs=2))       # small
    pm = ctx.enter_context(tc.tile_pool(name="pm", bufs=2, space="PSUM"))
    dp = ctx.enter_context(tc.tile_pool(name="dp", bufs=1, space="DRAM"))
    tc._dbg_pools = [wp, ap_, fp, sp, pm, dp]

    uid = [0]

    def nm(s):
        uid[0] += 1
        return f"{s}{uid[0]}"

    def psum_main(name):
        return pm.tile([128, 1024], F32, tag="pmain", bufs=3, name=nm(name))

    def psum_vec(name):
        return pm.tile([1, 1024], F32, tag="pvec", bufs=1, name=nm(name))

    # ---------------- small constants ----------------
    zc = sp.tile([128, 1], F32, tag="zc", bufs=1, name="zc")
    nc.vector.memset(zc, 0.0)
    epsct = sp.tile([128, 1], F32, tag="epsct", bufs=1, name="epsct")
    nc.vector.memset(epsct, 1e-5)
    # Packed small row-vectors. Compute engines only address partition bases
    # {0,32,64,96}, and 2-input DVE ops need EQUAL input bases — so each
    # phase gets one base (enc=0, dec1=32, dec2=64, layernorms=96) and its
    # vectors live at that base across three big tiles + a tiny scratch.
    svL = sp.tile([128, 1024], F32, tag="svL", bufs=1, name="svL")
    svX = sp.tile([128, 1024], F32, tag="svX", bufs=1, name="svX")
    svY = sp.tile([128, 1024], F32, tag="svY", bufs=1, name="svY")
    svS = sp.tile([128, 32], F32, tag="svS", bufs=1, name="svS")
    svh = sp.tile([128, 2048], F16, tag="svh", bufs=1, name="svh")
    PHASE_BASE = {0: 0, 1: 32, 2: 64}
    ones = sp.tile([128, 128], F16, tag="ones", bufs=1, name="ones")
    nc.sync.dma_start(ones, io["onesh"])
    # coef constants bounce through a DVE copy so downstream DVE ops that
    # also read DMA-produced operands don't exceed per-inst sync-wait slots
    coefLd = sp.tile([128, 12], F32, tag="coefLd", bufs=1, name="coefLd")
    nc.sync.dma_start(coefLd, io["coefL"])
    coefL = sp.tile([128, 12], F32, tag="coefL", bufs=1, name="coefL")
    nc.vector.tensor_copy(coefL, coefLd)
    coefRd = sp.tile([128, 12], F32, tag="coefRd", bufs=1, name="coefRd")
    nc.sync.dma_start(coefRd, io["coefR"])
    coefR = sp.tile([128, 12], F32, tag="coefR", bufs=1, name="coefR")
    nc.vector.tensor_copy(coefR, coefRd)
    bemb = sp.tile([128, NT], F32, tag="bemb", bufs=1, name="bemb")
    nc.sync.dma_start(bemb, io["bemb"])
    bfin = sp.tile([128, INF], F32, tag="bfin", bufs=1, name="bfin")
    nc.sync.dma_start(bfin, io["bfin"])
    lnw = {}
    for pfx in ("e", "d"):
        for wn in ("lng", "lnb"):
            t = sp.tile([128, NT], F32, tag=f"{pfx}{wn}", bufs=1,
                        name=f"{pfx}{wn}")
            nc.sync.dma_start(t, io[f"{pfx}_{wn}"])
            lnw[f"{pfx}{wn}"] = t

    # ---------------- helpers ----------------
    def load_proj_w(name):
        """[HID, ncol] fp16 dram -> NT tiles [128, ncol]."""
        ncol = io[name].shape[1]
        ts = []
        for ci in range(NT):
            t = wp.tile([128, ncol], F16, tag="pw", bufs=8, name=nm(f"{name}_"))
            nc.sync.dma_start(t, io[name][ci * 128:(ci + 1) * 128, :])
            ts.append(t)
        return ts

    def proj_cm(act_h, wts, evict, nci=NT, nco=HID):
        """out[co, l] = sum_ci w[ci, co] act[ci, l]; evict(cc, psum)."""
        for cc in range(nco // 128):
            ps = psum_main("pj")
            for ci in range(nci):
                for h in range(2):
                    nc.tensor.matmul(
                        ps[:, h * 512:(h + 1) * 512],
                        lhsT=wts[ci][:, cc * 128:(cc + 1) * 128],
                        rhs=act_h[ci][:, h * 512:(h + 1) * 512],
                        start=(ci == 0), stop=(ci == nci - 1))
            evict(cc, ps)

    def decomp(a_ap, s_out, t_out=None, cols=L):
        """seasonal/trend decomposition along free dim of one tile.
        a_ap: [P, cols] (fp16 or f32). s_out: seasonal dest (may be None).
        t_out: (dest, scale_into) trend dest fp16 if wanted."""
        cs = fp.tile([128, 1024], F32, tag="cs", bufs=1, name=nm("cs"))
        cs = cs[:a_ap.shape[0], :cols]
        nc.vector.tensor_tensor_scan(
            cs, a_ap, zc[:a_ap.shape[0], :].to_broadcast([a_ap.shape[0], cols]),
            0.0, ALU.add, ALU.add)
        d = fp.tile([128, 1024], F32, tag="dwin", bufs=1, name=nm("dw"))
        d = d[:a_ap.shape[0], :cols]
        n = cols
        nc.vector.tensor_sub(d[:, 13:n - 12], cs[:, 25:n], cs[:, 0:n - 25])
        nc.vector.tensor_copy(d[:, 0:13], cs[:, 12:25])
        nc.vector.scalar_tensor_tensor(
            d[:, n - 12:n], in0=cs[:, n - 25:n - 13], scalar=-1.0,
            in1=cs[:, n - 1:n].to_broadcast([a_ap.shape[0], 12]),
            op0=ALU.mult, op1=ALU.add)
        nc.vector.scalar_tensor_tensor(
            d[:, 0:12], in0=coefL[:a_ap.shape[0], :], scalar=a_ap[:, 0:1],
            in1=d[:, 0:12], op0=ALU.mult, op1=ALU.add)
        nc.vector.scalar_tensor_tensor(
            d[:, n - 12:n], in0=coefR[:a_ap.shape[0], :],
            scalar=a_ap[:, n - 1:n],
            in1=d[:, n - 12:n], op0=ALU.mult, op1=ALU.add)
        if s_out is not None:
            nc.vector.scalar_tensor_tensor(
                s_out, in0=d, scalar=-1.0 / KD, in1=a_ap,
                op0=ALU.mult, op1=ALU.add)
        if t_out is not None:
            nc.vector.tensor_scalar_mul(t_out, d, 1.0 / KD)

    # ================= autocorrelation =================
    def ac_gram(q, k, phase):
        """q,k: NT fp16 tiles [128,1024] (channel-major). Computes local cvec
        and kicks off AllReduce. Returns (lv, gv_dram or None, gdram)."""
        gdram = dp.tile([L, 2 * L], F16, tag="gd", bufs=2, name=nm("gd"))
        for nt in range(8):
            ps = psum_main("gram")
            for ci in range(NT):
                for h in range(2):
                    nc.tensor.matmul(
                        ps[:, h * 512:(h + 1) * 512],
                        lhsT=q[ci][:, nt * 128:(nt + 1) * 128],
                        rhs=k[ci][:, h * 512:(h + 1) * 512],
                        start=(ci == 0), stop=(ci == NT - 1))
            gh = ap_.tile([128, 1024], F16, tag="gh", bufs=2, name=nm("gh"))
            nc.scalar.copy(gh, ps)
            nc.sync.dma_start(gdram[nt * 128:(nt + 1) * 128, 0:L], gh)
            nc.sync.dma_start(gdram[nt * 128:(nt + 1) * 128, L:2 * L], gh)
        pv = psum_vec("cv")
        for nt in range(8):
            dt = ap_.tile([128, 1024], F16, tag="dg", bufs=2, name=nm("dg"))
            src = bass.AP(gdram.tensor, 2049 * 128 * nt + 1,
                          [[2049, 128], [1, 1024]])
            nc.sync.dma_start(dt, src)
            for h in range(2):
                nc.tensor.matmul(pv[:, h * 512:(h + 1) * 512],
                                 lhsT=ones[:, 0:1],
                                 rhs=dt[:, h * 512:(h + 1) * 512],
                                 start=(nt == 0), stop=(nt == 7))
        b = PHASE_BASE[phase]
        lv = svL[b:b + 1, :]
        nc.vector.tensor_copy(lv, pv)
        if n_cores > 1:
            cci = dp.tile([1, 1024], F32, tag="cci", bufs=2, name=nm("cci"))
            cco = dp.tile([1, 1024], F32, tag="cco", bufs=2,
                          addr_space="Shared", name=nm("cco"))
            nc.sync.dma_start(cci, lv)
            nc.gpsimd.collective_compute(
                "AllReduce", ALU.add,
                replica_groups=[list(range(n_cores))],
                ins=[cci.opt()], outs=[cco.opt()])
            gv = svX[b:b + 1, :]
            nc.sync.dma_start(gv, cco)
        else:
            gv = lv
        return lv, gv

    def ac_weights(lv, gv, phase):
        """top-20 mask from gv, softmax of lv/HID over mask -> circulant
        weights written duplicated to DRAM. Returns wd dram tile.
        All vectors at this phase's partition base; X/Y buffers rotate."""
        b = PHASE_BASE[phase]
        S = svS[b:b + 1, :]
        X = svX[b:b + 1, :]
        Y = svY[b:b + 1, :]
        m8, m8b, m8c = S[:, 0:8], S[:, 8:16], S[:, 16:24]
        mx, mxn, sm, rc = (S[:, 24:25], S[:, 25:26], S[:, 26:27], S[:, 27:28])
        nc.vector.max(m8, gv)
        nc.vector.match_replace(Y, in_to_replace=m8, in_values=gv,
                                imm_value=NEG)                    # cur1 = Y
        nc.vector.max(m8b, Y)
        nc.vector.match_replace(X, in_to_replace=m8b, in_values=Y,
                                imm_value=NEG)                    # cur2 = X
        nc.vector.max(m8c, X)
        nc.vector.memset(m8c[:, TOPK - 16:8], NEG)
        nc.vector.match_replace(Y, in_to_replace=m8c, in_values=X,
                                imm_value=NEG)                    # cur3 = Y
        # masked logits: msk*(lv - SMALLNEG) + SMALLNEG
        nc.vector.tensor_scalar(X, Y, float(NEG), None, op0=ALU.is_le)  # msk
        nc.vector.tensor_scalar_add(Y, lv, -float(SMALLNEG))           # ml0
        nc.vector.tensor_mul(X, X, Y)
        nc.vector.tensor_scalar_add(Y, X, float(SMALLNEG))             # ml
        nc.vector.reduce_max(mx, Y, axis=mybir.AxisListType.X)
        nc.vector.tensor_scalar_mul(mxn, mx, -1.0 / HID)
        nc.scalar.activation(X, Y, AF.Exp, bias=mxn, scale=1.0 / HID)  # ex
        nc.vector.reduce_sum(sm, X, axis=mybir.AxisListType.X)
        nc.vector.reciprocal(rc, sm)
        nc.vector.tensor_scalar_mul(Y, X, rc)                          # w
        # circulant source buffer: B[j] = W[(j-1) mod 1024], length 129*1024.
        # R_mt[p, l] = W[(l - (128*mt+p) + 1023) mod 1024] = B[(1024-128*mt)
        # + 1023*p + l] — all-positive strides for the DMA.
        wrot = svh[32 * (phase + 1):32 * (phase + 1) + 1, 0:1024]
        nc.scalar.copy(wrot[:, 0:1], Y[:, 1023:1024])
        nc.scalar.copy(wrot[:, 1:1024], Y[:, 0:1023])
        wd = dp.tile([129 * 1024], F16, tag="wd", bufs=2, name=nm("wd"))
        src = wrot.unsqueeze(1).to_broadcast([1, 129, 1024])
        nc.sync.dma_start(wd, src)
        return wd

    def ac_apply(wd, v, wo_ts, resid_h, a_out_tiles):
        """circulant matmul with v (seq-major tiles), then wo projection with
        residual add. a_out_tiles: list of 4 dest tiles (fp16) for
        wo_out + resid."""
        agg = []
        for cp in range(0, NT, 2):
            pss = [psum_main("circ"), psum_main("circ")]
            for mt in range(8):
                rt = ap_.tile([128, 1024], F16, tag="rt", bufs=2, name=nm("rt"))
                src = bass.AP(wd.tensor, 1024 - 128 * mt,
                              [[1023, 128], [1, 1024]])
                nc.sync.dma_start(rt, src)
                for j, ps in enumerate(pss):
                    cc = cp + j
                    for h in range(2):
                        nc.tensor.matmul(
                            ps[:, h * 512:(h + 1) * 512],
                            lhsT=v[mt][:, cc * 128:(cc + 1) * 128],
                            rhs=rt[:, h * 512:(h + 1) * 512],
                            start=(mt == 0), stop=(mt == 7))
            for j, ps in enumerate(pss):
                ag = ap_.tile([128, 1024], F16, tag=f"th{cp + j}", bufs=1,
                              name=nm("agg"))
                nc.scalar.copy(ag, ps)
                agg.append(ag)

        def evict(cc, ps):
            nc.vector.tensor_add(a_out_tiles[cc], ps, resid_h[cc])
        proj_cm(agg, wo_ts, evict)

    def proj_sm(act_h, wts, tag):
        """v seq-major: out[l, co] = sum_ci act[ci, l-chunk]^T w[ci, co].
        Returns 8 tiles [128, 512] fp16."""
        vts = []
        for lt in range(8):
            ps = pm.tile([128, 512], F32, tag="pmain", bufs=3, name=nm("pv"))
            for ci in range(NT):
                nc.tensor.matmul(ps,
                                 lhsT=act_h[ci][:, lt * 128:(lt + 1) * 128],
                                 rhs=wts[ci],
                                 start=(ci == 0), stop=(ci == NT - 1))
            vt = ap_.tile([128, 512], F16, tag=f"v{lt}", bufs=1, name=nm("v"))
            nc.scalar.copy(vt, ps)
            vts.append(vt)
        return vts

    # ================= conv block =================
    def leaky_evict(dst_ap, ps_ap, n):
        """dst = leaky_relu(ps) = max(ps, 0.01*ps), via fp16 tmp."""
        tmp = ap_.tile([128, 512], F16, tag="lkt", bufs=3, name=nm("lkt"))
        nc.scalar.copy(tmp[:, 0:n], ps_ap)
        nc.vector.scalar_tensor_tensor(
            dst_ap, in0=tmp[:, 0:n], scalar=0.01, in1=tmp[:, 0:n],
            op0=ALU.mult, op1=ALU.max)

    def conv_block(sptiles, w1name, w2name, cf_tag):
        """sptiles: 4 fp16 tiles [128, 1026] (replicate-padded seasonal).
        Returns 4 fp16 tiles [128, 1024]: leaky(conv2(leaky(conv1(s)))) + s."""
        w1, w2 = io[w1name], io[w2name]
        cf = [ap_.tile([128, 1024], F16, tag=f"{cf_tag}{i}", bufs=1,
                       name=nm("cf")) for i in range(NT)]
        for lh in range(2):
            c1h = []
            for co in range(CT):
                w1t = []
                for ci in range(NT):
                    t = wp.tile([128, 384], F16, tag=f"w1_{ci}", bufs=2,
                                name=nm("w1t"))
                    r0 = co * HID + ci * 128
                    nc.sync.dma_start(t, w1[r0:r0 + 128, :])
                    w1t.append(t)
                # 513 true outputs j in [lh*511, lh*511 + 513), computed as
                # 257 cols in psum bank 0 + 256 cols at bank-1 start (col 512)
                # so no matmul crosses a psum bank boundary
                ps = pm.tile([128, 1024], F32, tag="pmain", bufs=3,
                             name=nm("pc1"))
                j0 = 511 * lh
                for ci in range(NT):
                    for tp in range(3):
                        first = (ci == 0 and tp == 0)
                        last = (ci == NT - 1 and tp == 2)
                        for (o0, j1, n) in ((0, 0, 257), (512, 257, 256)):
                            nc.tensor.matmul(
                                ps[:, o0:o0 + n],
                                lhsT=w1t[ci][:, tp * 128:(tp + 1) * 128],
                                rhs=sptiles[ci][:, j0 + j1 + tp:
                                                j0 + j1 + tp + n],
                                start=first, stop=last)
                ch = ap_.tile([128, 514], F16, tag=f"c1_{co}", bufs=1,
                              name=nm("c1h"))
                # tile col t holds output j = lh*511 + t - (1-lh)
                if lh == 0:
                    leaky_evict(ch[:, 1:258], ps[:, 0:257], 257)
                    leaky_evict(ch[:, 258:514], ps[:, 512:768], 256)
                    nc.vector.tensor_copy(ch[:, 0:1], ch[:, 1:2])
                else:
                    leaky_evict(ch[:, 0:257], ps[:, 0:257], 257)
                    leaky_evict(ch[:, 257:513], ps[:, 512:768], 256)
                    nc.vector.tensor_copy(ch[:, 513:514], ch[:, 512:513])
                c1h.append(ch)
            # conv2 for this half: out cols [lh*512, lh*512+512)
            for co in range(NT):
                w2t = []
                for ci in range(CT):
                    t = wp.tile([128, 384], F16, tag=f"w2_{ci}", bufs=1,
                                name=nm("w2t"))
                    r0 = co * CF + ci * 128
                    nc.sync.dma_start(t, w2[r0:r0 + 128, :])
                    w2t.append(t)
                ps = pm.tile([128, 512], F32, tag="pmain", bufs=3,
                             name=nm("pc2"))
                # c2 out col l = lh*512 + t needs c1[l + tp - 1], which lives
                # at c1h col t + tp for both halves
                for ci in range(CT):
                    for tp in range(3):
                        nc.tensor.matmul(
                            ps,
                            lhsT=w2t[ci][:, tp * 128:(tp + 1) * 128],
                            rhs=c1h[ci][:, tp:tp + 512],
                            start=(ci == 0 and tp == 0),
                            stop=(ci == CT - 1 and tp == 2))
                tmp = ap_.tile([128, 512], F16, tag="c2t", bufs=2,
                               name=nm("c2t"))
                leaky_evict(tmp, ps, 512)
                nc.vector.tensor_add(
                    cf[co][:, lh * 512:(lh + 1) * 512], tmp,
                    sptiles[co][:, lh * 512 + 1:lh * 512 + 513])
        return cf

    # ================= layer norm =================
    def layer_norm(x_h, g_ap, b_ap, out_tiles, lnphase):
        """x_h: 4 fp16 tiles [128,1024] channel-major. out: 4 fp16 tiles."""
        psx = psum_vec("lnx")
        psq = psum_vec("lnq")
        for ci in range(NT):
            sq = ap_.tile([128, 1024], F16, tag="sqh", bufs=2, name=nm("sq"))
            nc.scalar.square(sq, x_h[ci])
            for h in range(2):
                nc.tensor.matmul(psx[:, h * 512:(h + 1) * 512],
                                 lhsT=ones[:, 0:1],
                                 rhs=x_h[ci][:, h * 512:(h + 1) * 512],
                                 start=(ci == 0), stop=(ci == NT - 1))
                nc.tensor.matmul(psq[:, h * 512:(h + 1) * 512],
                                 lhsT=ones[:, 0:1],
                                 rhs=sq[:, h * 512:(h + 1) * 512],
                                 start=(ci == 0), stop=(ci == NT - 1))
        mean = svL[96:97, :]
        nc.vector.tensor_scalar_mul(mean, psx, 1.0 / HID)
        var = svX[96:97, :]
        # var = psq/HID - mean^2
        nc.vector.tensor_mul(var, mean, mean)
        nc.vector.scalar_tensor_tensor(var, in0=psq, scalar=1.0 / HID,
                                       in1=var, op0=ALU.mult, op1=ALU.subtract)
        std = svY[96:97, :]
        nc.scalar.activation(std, var, AF.Sqrt, bias=epsct[96:97, :])
        rstd = svX[96:97, :]   # var dead
        nc.vector.reciprocal(rstd, std)
        mh = svh[0:1, 0:1024]
        nc.vector.tensor_copy(mh, mean)
        rh = svh[0:1, 1024:2048]
        nc.vector.tensor_copy(rh, rstd)
        # broadcast via rank-1 matmul
        psb_m = psum_main("bcm")
        psb_r = psum_main("bcr")
        for h in range(2):
            nc.tensor.matmul(psb_m[:, h * 512:(h + 1) * 512],
                             lhsT=ones[0:1, 0:128],
                             rhs=mh[:, h * 512:(h + 1) * 512],
                             start=True, stop=True)
            nc.tensor.matmul(psb_r[:, h * 512:(h + 1) * 512],
                             lhsT=ones[0:1, 0:128],
                             rhs=rh[:, h * 512:(h + 1) * 512],
                             start=True, stop=True)
        for ci in range(NT):
            t1 = ap_.tile([128, 1024], F16, tag="lnt", bufs=2, name=nm("lnt"))
            nc.vector.tensor_sub(t1, x_h[ci], psb_m)
            nc.vector.tensor_mul(t1, t1, psb_r)
            nc.vector.tensor_scalar(out_tiles[ci], t1,
                                    g_ap[:, ci:ci + 1], b_ap[:, ci:ci + 1],
                                    op0=ALU.mult, op1=ALU.add)

    # ======================================================================
    # phase A — input prep + embeddings
    # ======================================================================
    with nc.named_scope("prep"):
        xtd = fp.tile([INF, 1024], F32, tag="cs", bufs=1, name="xtd")
        nc.sync.dma_start(xtd, io["xt"])
        # DVE bounce: gives scan/stt consumers a same-engine producer
        xtf = fp.tile([INF, 1024], F32, tag="af", bufs=2, name="xtf")
        nc.vector.tensor_copy(xtf, xtd)
        xth = ap_.tile([INF, 1024], F16, tag="lnt", bufs=2, name="xth")
        nc.scalar.copy(xth, xtf)

    # initial decomposition: need the window-sum tensor D directly (both
    # seasonal and trend halves are consumed), so inline it here
    with nc.named_scope("initdecomp"):
        dx = fp.tile([INF, 1024], F32, tag="dwin", bufs=1, name="dx")
        cs0 = fp.tile([128, 1024], F32, tag="cs", bufs=1, name="cs0")
        cs0 = cs0[:INF, :]
        nc.vector.tensor_tensor_scan(
            cs0, xtf, zc[:INF, :].to_broadcast([INF, 1024]),
            0.0, ALU.add, ALU.add)
        d0 = dx
        nc.vector.tensor_sub(d0[:, 13:1012], cs0[:, 25:1024], cs0[:, 0:999])
        nc.vector.tensor_copy(d0[:, 0:13], cs0[:, 12:25])
        nc.vector.scalar_tensor_tensor(
            d0[:, 1012:1024], in0=cs0[:, 999:1011], scalar=-1.0,
            in1=cs0[:, 1023:1024].to_broadcast([INF, 12]),
            op0=ALU.mult, op1=ALU.add)
        nc.vector.scalar_tensor_tensor(
            d0[:, 0:12], in0=coefL[:INF, :], scalar=xtf[:, 0:1],
            in1=d0[:, 0:12], op0=ALU.mult, op1=ALU.add)
        nc.vector.scalar_tensor_tensor(
            d0[:, 1012:1024], in0=coefR[:INF, :], scalar=xtf[:, 1023:1024],
            in1=d0[:, 1012:1024], op0=ALU.mult, op1=ALU.add)

        # seasonal_init: cols 0:512 = (x - D/25)[:, 512:], cols 512: = 0
        sih = ap_.tile([INF, 1024], F16, tag="sqh", bufs=2, name="sih")
        nc.vector.scalar_tensor_tensor(
            sih[:, 0:512], in0=d0[:, 512:1024], scalar=-1.0 / KD,
            in1=xtf[:, 512:1024], op0=ALU.mult, op1=ALU.add)
        nc.vector.memset(sih[:, 512:1024], 0.0)
        # trend_init: cols 0:512 = D[:, 512:]/25, cols 512: = mean(x)
        tih = ap_.tile([INF, 1024], F16, tag="sqh", bufs=2, name="tih")
        nc.vector.tensor_scalar_mul(tih[:, 0:512], d0[:, 512:1024], 1.0 / KD)
        mnx = sp.tile([INF, 1], F32, tag="mnx", bufs=1, name="mnx")
        nc.vector.reduce_sum(mnx, xtf, axis=mybir.AxisListType.X)
        nc.vector.tensor_scalar_mul(mnx, mnx, 1.0 / L)
        nc.vector.tensor_scalar(
            tih[:, 512:1024],
            zc[:INF, :].to_broadcast([INF, 512]), mnx, None,
            op0=ALU.add)

    with nc.named_scope("embed"):
        wemb = [wp.tile([INF, HID], F16, tag="pw", bufs=8, name="wemb")]
        nc.sync.dma_start(wemb[0], io["we"])

        def emb(src_h, dst_tag, f32_dst=False, bufs=1):
            outs = []
            for cc in range(NT):
                ps = psum_main("emb")
                for h in range(2):
                    nc.tensor.matmul(
                        ps[:, h * 512:(h + 1) * 512],
                        lhsT=wemb[0][:, cc * 128:(cc + 1) * 128],
                        rhs=src_h[:, h * 512:(h + 1) * 512],
                        start=True, stop=True)
                if f32_dst:
                    o = fp.tile([128, 1024], F32, tag=f"{dst_tag}{cc}",
                                bufs=bufs, name=nm(dst_tag))
                    nc.vector.tensor_scalar_add(o, ps, bemb[:, cc:cc + 1])
                else:
                    o = ap_.tile([128, 1024], F16, tag=f"{dst_tag}{cc}",
                                 bufs=bufs, name=nm(dst_tag))
                    nc.scalar.activation(o, ps, AF.Identity,
                                         bias=bemb[:, cc:cc + 1])
                outs.append(o)
            return outs

        xe = emb(xth, "xe")
        se = emb(sih, "se")
        trend = emb(tih, "tr", f32_dst=True)  # running trend accumulator

    # ======================================================================
    # encoder autocorrelation (+ dec-ac1 gram interleaved to hide latency)
    # ======================================================================
    with nc.named_scope("enc_ac_gram"):
        wq = load_proj_w("e_wq")
        wk = load_proj_w("e_wk")
        q = []
        k = []
        for cc in range(NT):
            qt = ap_.tile([128, 1024], F16, tag=f"q{cc}", bufs=1, name=nm("q"))
            kt = ap_.tile([128, 1024], F16, tag=f"k{cc}", bufs=1, name=nm("k"))
            q.append(qt)
            k.append(kt)
        proj_cm(xe, wq, lambda cc, ps: nc.scalar.copy(q[cc], ps))
        proj_cm(xe, wk, lambda cc, ps: nc.scalar.copy(k[cc], ps))
        lv_e, gv_e = ac_gram(q, k, 0)

    with nc.named_scope("dec1_ac_gram"):
        wq1 = load_proj_w("d1_wq")
        wk1 = load_proj_w("d1_wk")
        q1 = []
        k1 = []
        for cc in range(NT):
            qt = ap_.tile([128, 1024], F16, tag=f"q{cc}", bufs=1, name=nm("q1"))
            kt = ap_.tile([128, 1024], F16, tag=f"k{cc}", bufs=1, name=nm("k1"))
            q1.append(qt)
            k1.append(kt)
        proj_cm(se, wq1, lambda cc, ps: nc.scalar.copy(q1[cc], ps))
        proj_cm(se, wk1, lambda cc, ps: nc.scalar.copy(k1[cc], ps))
        lv_1, gv_1 = ac_gram(q1, k1, 1)

    with nc.named_scope("enc_ac_apply"):
        wv_ = load_proj_w("e_wv")
        v = proj_sm(xe, wv_, "v")
        wd_e = ac_weights(lv_e, gv_e, 0)
        wo_ = load_proj_w("e_wo")
        a_enc = [fp.tile([128, 1024], F32, tag="af", bufs=2, name=nm("ae"))
                 for _ in range(NT)]
        ac_apply(wd_e, v, wo_, xe, a_enc)

    # ======================================================================
    # encoder decomp 1 -> s1 (padded), convs, decomp 2, LN
    # ======================================================================
    with nc.named_scope("enc_decomp1"):
        s1p = [ap_.tile([128, 1026], F16, tag=f"sp{i}", bufs=1, name=nm("s1p"))
               for i in range(NT)]
        for cc in range(NT):
            decomp(a_enc[cc], s1p[cc][:, 1:1025])
            nc.vector.tensor_copy(s1p[cc][:, 0:1], s1p[cc][:, 1:2])
            nc.vector.tensor_copy(s1p[cc][:, 1025:1026], s1p[cc][:, 1024:1025])

    with nc.named_scope("enc_convs"):
        cf_e = conv_block(s1p, "e_w1", "e_w2", "cf")

    with nc.named_scope("enc_ln"):
        sf = [ap_.tile([128, 1024], F16, tag=f"sf{i}", bufs=1, name=nm("sf"))
              for i in range(NT)]
        for cc in range(NT):
            decomp(cf_e[cc], sf[cc])
        enc_out = [ap_.tile([128, 1024], F16, tag=f"eo{i}", bufs=1,
                            name=nm("eo")) for i in range(NT)]
        layer_norm(sf, lnw["elng"], lnw["elnb"], enc_out, 0)

    # ======================================================================
    # decoder ac1 apply -> a1 -> decomp -> s1d, t1 -> lin1 -> trend
    # ======================================================================
    with nc.named_scope("dec1_apply"):
        wv1 = load_proj_w("d1_wv")
        v1 = proj_sm(se, wv1, "v")
        wd_1 = ac_weights(lv_1, gv_1, 1)
        wo1 = load_proj_w("d1_wo")
        a1 = [fp.tile([128, 1024], F32, tag="af", bufs=2, name=nm("a1"))
              for _ in range(NT)]
        ac_apply(wd_1, v1, wo1, se, a1)

    with nc.named_scope("dec1_decomp"):
        s1d = [ap_.tile([128, 1024], F16, tag=f"s1d{i}", bufs=1, name=nm("s1d"))
               for i in range(NT)]
        th = [ap_.tile([128, 1024], F16, tag=f"th{i}", bufs=1, name=nm("th"))
              for i in range(NT)]
        for cc in range(NT):
            decomp(a1[cc], s1d[cc], t_out=th[cc])
        wl1 = load_proj_w("lin1")

        def ev_t1(cc, ps):
            nc.vector.tensor_add(trend[cc], trend[cc], ps)
        proj_cm(th, wl1, ev_t1)

    # ======================================================================
    # decoder ac2: q from s1d, k/v from enc_out
    # ======================================================================
    with nc.named_scope("dec2_ac"):
        wq2 = load_proj_w("d2_wq")
        wk2 = load_proj_w("d2_wk")
        q2 = []
        k2 = []
        for cc in range(NT):
            qt = ap_.tile([128, 1024], F16, tag=f"q{cc}", bufs=1, name=nm("q2"))
            kt = ap_.tile([128, 1024], F16, tag=f"k{cc}", bufs=1, name=nm("k2"))
            q2.append(qt)
            k2.append(kt)
        proj_cm(s1d, wq2, lambda cc, ps: nc.scalar.copy(q2[cc], ps))
        proj_cm(enc_out, wk2, lambda cc, ps: nc.scalar.copy(k2[cc], ps))
        lv_2, gv_2 = ac_gram(q2, k2, 2)
        wv2 = load_proj_w("d2_wv")
        v2 = proj_sm(enc_out, wv2, "v")
        wd_2 = ac_weights(lv_2, gv_2, 2)
        wo2 = load_proj_w("d2_wo")
        a2 = [fp.tile([128, 1024], F32, tag="af", bufs=2, name=nm("a2"))
              for _ in range(NT)]
        ac_apply(wd_2, v2, wo2, s1d, a2)

    with nc.named_scope("dec2_decomp"):
        s2p = [ap_.tile([128, 1026], F16, tag=f"sp{i}", bufs=1, name=nm("s2p"))
               for i in range(NT)]
        th2 = [ap_.tile([128, 1024], F16, tag=f"th{i}", bufs=1, name=nm("th2"))
               for i in range(NT)]
        for cc in range(NT):
            decomp(a2[cc], s2p[cc][:, 1:1025], t_out=th2[cc])
            nc.vector.tensor_copy(s2p[cc][:, 0:1], s2p[cc][:, 1:2])
            nc.vector.tensor_copy(s2p[cc][:, 1025:1026], s2p[cc][:, 1024:1025])
        wl2 = load_proj_w("lin2")

        def ev_t2(cc, ps):
            nc.vector.tensor_add(trend[cc], trend[cc], ps)
        proj_cm(th2, wl2, ev_t2)

    # ======================================================================
    # decoder convs, decomp 3, LN, season + trend, final projection
    # ======================================================================
    with nc.named_scope("dec_convs"):
        cf_d = conv_block(s2p, "d_w1", "d_w2", "cf")

    with nc.named_scope("dec_final"):
        s3 = [ap_.tile([128, 1024], F16, tag=f"sf{i}", bufs=1, name=nm("s3"))
              for i in range(NT)]
        th3 = [ap_.tile([128, 1024], F16, tag=f"th{i}", bufs=1, name=nm("th3"))
               for i in range(NT)]
        for cc in range(NT):
            decomp(cf_d[cc], s3[cc], t_out=th3[cc])
        wl3 = load_proj_w("lin3")

        def ev_t3(cc, ps):
            nc.vector.tensor_add(trend[cc], trend[cc], ps)
        proj_cm(th3, wl3, ev_t3)

        sea = [ap_.tile([128, 1024], F16, tag=f"xe{i}", bufs=1, name=nm("sea"))
               for i in range(NT)]
        layer_norm(s3, lnw["dlng"], lnw["dlnb"], sea, 1)
        wls = load_proj_w("lins")
        fin = [ap_.tile([128, 1024], F16, tag=f"q{i}", bufs=1,
                        name=nm("fin")) for i in range(NT)]

        def ev_sea(cc, ps):
            nc.vector.tensor_add(fin[cc], ps, trend[cc])
        proj_cm(sea, wls, ev_sea)

        # final: out[l, co] for l in [512, 1024)
        wfin = [wp.tile([128, INF], F16, tag="pw", bufs=8, name=nm("wfin"))
                for _ in range(NT)]
        for ci in range(NT):
            nc.sync.dma_start(wfin[ci], io["wf"][ci * 128:(ci + 1) * 128, :])
        for lt in range(4, 8):
            ps = pm.tile([128, INF], F32, tag="pmain", bufs=3, name=nm("pf"))
            for ci in range(NT):
                nc.tensor.matmul(ps,
                                 lhsT=fin[ci][:, lt * 128:(lt + 1) * 128],
                                 rhs=wfin[ci],
                                 start=(ci == 0), stop=(ci == NT - 1))
            of = sp.tile([128, INF], F32, tag="of", bufs=2, name=nm("of"))
            nc.vector.tensor_add(of, ps, bfin)
            nc.sync.dma_start(
                out[(lt - 4) * 128:(lt - 3) * 128, :], of)

    ctx.close()


# --------------------------------------------------------------------------
# host driver
# --------------------------------------------------------------------------

def _prep_inputs(x, params):
    """Returns list of per-core in_maps."""
    g = lambda *ks: np.asarray(_dig(params, ks))
    shared = {}
    shared["we"] = g("w_emb").astype(np.float16)
    shared["wf"] = g("w_final").astype(np.float16)
    shared["bemb"] = np.ascontiguousarray(
        g("b_emb").reshape(NT, 128).T.astype(np.float32))
    bf = g("b_final").astype(np.float32)
    shared["bfin"] = np.ascontiguousarray(
        np.broadcast_to(bf[None, :], (128, INF)).copy())
    shared["coefL"] = np.ascontiguousarray(np.broadcast_to(
        np.arange(12, 0, -1, dtype=np.float32)[None, :], (128, 12)).copy())
    shared["coefR"] = np.ascontiguousarray(np.broadcast_to(
        np.arange(1, 13, dtype=np.float32)[None, :], (128, 12)).copy())
    shared["onesh"] = np.ones((128, 128), np.float16)
    acmap = {"e": ("enc", "ac"), "d1": ("dec", "ac1"), "d2": ("dec", "ac2")}
    for pfx, ks in acmap.items():
        for wn in ("wq", "wk", "wv", "wo"):
            shared[f"{pfx}_{wn}"] = g(*ks, wn).astype(np.float16)
    for i, wn in enumerate(("lin1", "lin2", "lin3")):
        shared[wn] = g("dec", wn).astype(np.float16)
    shared["lins"] = g("dec", "lin_season").astype(np.float16)
    for pfx, side in (("e", "enc"), ("d", "dec")):
        w1 = g(side, "conv1")  # [CF, HID, 3]
        w1r = w1.reshape(CT, 128, HID, 3).transpose(0, 2, 3, 1)
        shared[f"{pfx}_w1"] = np.ascontiguousarray(
            w1r.reshape(CT * HID, 384)).astype(np.float16)
        w2 = g(side, "conv2")  # [HID, CF, 3]
        w2r = w2.reshape(NT, 128, CF, 3).transpose(0, 2, 3, 1)
        shared[f"{pfx}_w2"] = np.ascontiguousarray(
            w2r.reshape(NT * CF, 384)).astype(np.float16)
        shared[f"{pfx}_lng"] = np.ascontiguousarray(
            g(side, "ln_g").reshape(NT, 128).T.astype(np.float32))
        shared[f"{pfx}_lnb"] = np.ascontiguousarray(
            g(side, "ln_b").reshape(NT, 128).T.astype(np.float32))

    in_maps = []
    for c in range(N_CORES):
        m = dict(shared)
        m["xt"] = np.ascontiguousarray(np.asarray(x[c]).T.astype(np.float32))
        in_maps.append(m)
    return in_maps


def _dig(d, ks):
    for k in ks:
        d = d[k]
    return d


def kernel(x, params):
    global LAST_RESULT, _CACHED
    import os
    try:
        import antenv.axon_hooks  # noqa: F401
    except ImportError:
        # tracing under axon needs this hook; without it a stray BASS_TRACE
        # in the environment would crash the run
        os.environ["BASS_NEVER_TRACE"] = "1"
    x = np.asarray(x)
    if _CACHED is None:
        _CACHED = build_program(N_CORES)
    nc = _CACHED
    in_maps = _prep_inputs(x, params)
    res = run_bass_kernel_spmd(nc, in_maps, core_ids=list(range(N_CORES)))
    LAST_RESULT = res
    outs = np.stack([r["out"] for r in res.results], axis=0)
    return outs.astype(np.float32)


if __name__ == "__main__":
    nc = build_program(1)
    print("program built OK")


# revision 30
# speedup vs baseline: 1.0292x; 1.0017x over previous
"""AutoFormer forward pass on 8 Trainium2 NeuronCores (Bass/Tile).

Data-parallel over batch (1 sample per core). All heavy compute in fp16
matmuls on the PE (fp32 PSUM accumulate); vector math in fp32.

Key tricks:
  - Activations kept channel-major [C, L] so every projection/conv is a
    natural PE matmul with pre-transposed weights.
  - AutoCorrelation without FFT: Gram matrix G = q^T k (PE), circular
    diagonal sums extracted with a stride-2049 DMA from a column-duplicated
    DRAM copy of G, reduced with ones-matmuls -> unnormalized mean_corr.
  - Shared top-k delays need the batch mean -> one tiny [1024] fp32
    AllReduce per autocorrelation (3 total).
  - top-20 mask via vector.max + match_replace (no indices anywhere).
  - Delay aggregation as a circulant matmul: softmax weights written to
    DRAM duplicated, re-read as a [128,1024] moving operand with
    per-partition descending offsets (circulant materialization by DMA).
  - Moving average (kernel 25, replicate pad) via fp32 prefix-scan
    (tensor_tensor_scan) + shifted window subtraction + rank-1 edge fixes.
"""

import sys

if "/opt/trn_rl_repo" not in sys.path:
    sys.path.insert(0, "/opt/trn_rl_repo")

import numpy as np

import concourse.bass as bass
import concourse.mybir as mybir
import concourse.tile as tile
from concourse import bacc
from concourse.bass_utils import run_bass_kernel_spmd

F16 = mybir.dt.float16
F32 = mybir.dt.float32
AF = mybir.ActivationFunctionType
ALU = mybir.AluOpType

N_CORES = 8
L = 1024
HID = 512
INF = 64
CF = 2048
KD = 25
TOPK = 20
NT = HID // 128   # 4 channel tiles
CT = CF // 128    # 16 conv-mid channel tiles
NEG = -1e30
SMALLNEG = -1e5

LAST_RESULT = None
_CACHED = None


# --------------------------------------------------------------------------
# program builder
# --------------------------------------------------------------------------

def build_program(n_cores=N_CORES):
    nc = bacc.Bacc("TRN2", target_bir_lowering=False, debug=False,
                   num_devices=n_cores)

    io = {}

    def inp(name, shape, dtype):
        io[name] = nc.dram_tensor(name, list(shape), dtype,
                                  kind="ExternalInput").ap()
        return io[name]

    xt = inp("xt", [INF, L], F32)
    we = inp("we", [INF, HID], F16)
    wf = inp("wf", [HID, INF], F16)
    bemb = inp("bemb", [128, NT], F32)
    bfin = inp("bfin", [128, INF], F32)
    coefL = inp("coefL", [128, 12], F32)
    coefR = inp("coefR", [128, 12], F32)
    onesh = inp("onesh", [128, 128], F16)
    # per-autocorr projection weights [HID, HID] fp16, natural [cin, cout]
    for pfx in ("e", "d1", "d2"):
        for wn in ("wq", "wk", "wv", "wo"):
            inp(f"{pfx}_{wn}", [HID, HID], F16)
    for wn in ("lin1", "lin2", "lin3", "lins"):
        inp(wn, [HID, HID], F16)
    for pfx in ("e", "d"):
        # conv weights pre-gathered per 128-wide output chunk so each
        # [128,384] SBUF tile is one DMA of contiguous 768B rows
        inp(f"{pfx}_w1", [CT * HID, 384], F16)   # [co_chunk, cin, tap*128]
        inp(f"{pfx}_w2", [NT * CF, 384], F16)
        inp(f"{pfx}_lng", [128, NT], F32)
        inp(f"{pfx}_lnb", [128, NT], F32)

    out = nc.dram_tensor("out", [512, INF], F32, kind="ExternalOutput").ap()

    with tile.TileContext(nc) as tc:
        _body(tc, io, out, n_cores)
    nc.compile()
    return nc


def _body(tc, io, out, n_cores):
    nc = tc.nc
    import contextlib
    ctx = contextlib.ExitStack()

    wp = ctx.enter_context(tc.tile_pool(name="wp", bufs=1))       # weights
    ap_ = ctx.enter_context(tc.tile_pool(name="ap", bufs=1))      # activations
    fp = ctx.enter_context(tc.tile_pool(name="fp", bufs=1))       # f32 work
    sp = ctx.enter_context(tc.tile_pool(name="sp", bufs=2))       # small
    pm = ctx.enter_context(tc.tile_pool(name="pm", bufs=2, space="PSUM"))
    dp = ctx.enter_context(tc.tile_pool(name="dp", bufs=1, space="DRAM"))
    tc._dbg_pools = [wp, ap_, fp, sp, pm, dp]

    uid = [0]

    def nm(s):
        uid[0] += 1
        return f"{s}{uid[0]}"

    def psum_main(name):
        return pm.tile([128, 1024], F32, tag="pmain", bufs=3, name=nm(name))

    def psum_vec(name):
        return pm.tile([1, 1024], F32, tag="pvec", bufs=1, name=nm(name))

    # ---------------- small constants ----------------
    zc = sp.tile([128, 1], F32, tag="zc", bufs=1, name="zc")
    nc.vector.memset(zc, 0.0)
    epsct = sp.tile([128, 1], F32, tag="epsct", bufs=1, name="epsct")
    nc.vector.memset(epsct, 1e-5)
    # Packed small row-vectors. Compute engines only address partition bases
    # {0,32,64,96}, and 2-input DVE ops need EQUAL input bases — so each
    # phase gets one base (enc=0, dec1=32, dec2=64, layernorms=96) and its
    # vectors live at that base across three big tiles + a tiny scratch.
    svL = sp.tile([128, 1024], F32, tag="svL", bufs=1, name="svL")
    svX = sp.tile([128, 1024], F32, tag="svX", bufs=1, name="svX")
    svY = sp.tile([128, 1024], F32, tag="svY", bufs=1, name="svY")
    svS = sp.tile([128, 32], F32, tag="svS", bufs=1, name="svS")
    svh = sp.tile([128, 2048], F16, tag="svh", bufs=1, name="svh")
    PHASE_BASE = {0: 0, 1: 32, 2: 64}
    ones = sp.tile([128, 128], F16, tag="ones", bufs=1, name="ones")
    nc.sync.dma_start(ones, io["onesh"])
    # coef constants bounce through a DVE copy so downstream DVE ops that
    # also read DMA-produced operands don't exceed per-inst sync-wait slots
    coefLd = sp.tile([128, 12], F32, tag="coefLd", bufs=1, name="coefLd")
    nc.sync.dma_start(coefLd, io["coefL"])
    coefL = sp.tile([128, 12], F32, tag="coefL", bufs=1, name="coefL")
    nc.vector.tensor_copy(coefL, coefLd)
    coefRd = sp.tile([128, 12], F32, tag="coefRd", bufs=1, name="coefRd")
    nc.sync.dma_start(coefRd, io["coefR"])
    coefR = sp.tile([128, 12], F32, tag="coefR", bufs=1, name="coefR")
    nc.vector.tensor_copy(coefR, coefRd)
    bemb = sp.tile([128, NT], F32, tag="bemb", bufs=1, name="bemb")
    nc.sync.dma_start(bemb, io["bemb"])
    bfin = sp.tile([128, INF], F32, tag="bfin", bufs=1, name="bfin")
    nc.sync.dma_start(bfin, io["bfin"])
    lnw = {}
    for pfx in ("e", "d"):
        for wn in ("lng", "lnb"):
            t = sp.tile([128, NT], F32, tag=f"{pfx}{wn}", bufs=1,
                        name=f"{pfx}{wn}")
            nc.sync.dma_start(t, io[f"{pfx}_{wn}"])
            lnw[f"{pfx}{wn}"] = t

    # ---------------- helpers ----------------
    def load_proj_w(name):
        """[HID, ncol] fp16 dram -> NT tiles [128, ncol]."""
        ncol = io[name].shape[1]
        ts = []
        for ci in range(NT):
            t = wp.tile([128, ncol], F16, tag="pw", bufs=8, name=nm(f"{name}_"))
            nc.sync.dma_start(t, io[name][ci * 128:(ci + 1) * 128, :])
            ts.append(t)
        return ts

    def proj_cm(act_h, wts, evict, nci=NT, nco=HID):
        """out[co, l] = sum_ci w[ci, co] act[ci, l]; evict(cc, psum)."""
        for cc in range(nco // 128):
            ps = psum_main("pj")
            for ci in range(nci):
                for h in range(2):
                    nc.tensor.matmul(
                        ps[:, h * 512:(h + 1) * 512],
                        lhsT=wts[ci][:, cc * 128:(cc + 1) * 128],
                        rhs=act_h[ci][:, h * 512:(h + 1) * 512],
                        start=(ci == 0), stop=(ci == nci - 1))
            evict(cc, ps)

    def decomp(a_ap, s_out, t_out=None, cols=L):
        """seasonal/trend decomposition along free dim of one tile.
        a_ap: [P, cols] (fp16 or f32). s_out: seasonal dest (may be None).
        t_out: (dest, scale_into) trend dest fp16 if wanted."""
        cs = fp.tile([128, 1024], F32, tag="cs", bufs=1, name=nm("cs"))
        cs = cs[:a_ap.shape[0], :cols]
        nc.vector.tensor_tensor_scan(
            cs, a_ap, zc[:a_ap.shape[0], :].to_broadcast([a_ap.shape[0], cols]),
            0.0, ALU.add, ALU.add)
        d = fp.tile([128, 1024], F32, tag="dwin", bufs=1, name=nm("dw"))
        d = d[:a_ap.shape[0], :cols]
        n = cols
        nc.vector.tensor_sub(d[:, 13:n - 12], cs[:, 25:n], cs[:, 0:n - 25])
        nc.vector.tensor_copy(d[:, 0:13], cs[:, 12:25])
        nc.vector.scalar_tensor_tensor(
            d[:, n - 12:n], in0=cs[:, n - 25:n - 13], scalar=-1.0,
            in1=cs[:, n - 1:n].to_broadcast([a_ap.shape[0], 12]),
            op0=ALU.mult, op1=ALU.add)
        nc.vector.scalar_tensor_tensor(
            d[:, 0:12], in0=coefL[:a_ap.shape[0], :], scalar=a_ap[:, 0:1],
            in1=d[:, 0:12], op0=ALU.mult, op1=ALU.add)
        nc.vector.scalar_tensor_tensor(
            d[:, n - 12:n], in0=coefR[:a_ap.shape[0], :],
            scalar=a_ap[:, n - 1:n],
            in1=d[:, n - 12:n], op0=ALU.mult, op1=ALU.add)
        if s_out is not None:
            nc.vector.scalar_tensor_tensor(
                s_out, in0=d, scalar=-1.0 / KD, in1=a_ap,
                op0=ALU.mult, op1=ALU.add)
        if t_out is not None:
            nc.vector.tensor_scalar_mul(t_out, d, 1.0 / KD)

    # ================= autocorrelation =================
    def ac_gram(q, k, phase):
        """q,k: NT fp16 tiles [128,1024] (channel-major). Computes local cvec
        and kicks off AllReduce. Returns (lv, gv_dram or None, gdram)."""
        gdram = dp.tile([L, 2 * L], F16, tag="gd", bufs=2, name=nm("gd"))
        for nt in range(8):
            ps = psum_main("gram")
            for ci in range(NT):
                for h in range(2):
                    nc.tensor.matmul(
                        ps[:, h * 512:(h + 1) * 512],
                        lhsT=q[ci][:, nt * 128:(nt + 1) * 128],
                        rhs=k[ci][:, h * 512:(h + 1) * 512],
                        start=(ci == 0), stop=(ci == NT - 1))
            gh = ap_.tile([128, 1024], F16, tag="gh", bufs=2, name=nm("gh"))
            nc.scalar.copy(gh, ps)
            nc.sync.dma_start(gdram[nt * 128:(nt + 1) * 128, 0:L], gh)
            nc.sync.dma_start(gdram[nt * 128:(nt + 1) * 128, L:2 * L], gh)
        pv = psum_vec("cv")
        for nt in range(8):
            dt = ap_.tile([128, 1024], F16, tag="dg", bufs=2, name=nm("dg"))
            src = bass.AP(gdram.tensor, 2049 * 128 * nt + 1,
                          [[2049, 128], [1, 1024]])
            nc.sync.dma_start(dt, src)
            for h in range(2):
                nc.tensor.matmul(pv[:, h * 512:(h + 1) * 512],
                                 lhsT=ones[:, 0:1],
                                 rhs=dt[:, h * 512:(h + 1) * 512],
                                 start=(nt == 0), stop=(nt == 7))
        b = PHASE_BASE[phase]
        lv = svL[b:b + 1, :]
        nc.vector.tensor_copy(lv, pv)
        if n_cores > 1:
            cci = dp.tile([1, 1024], F32, tag="cci", bufs=2, name=nm("cci"))
            cco = dp.tile([1, 1024], F32, tag="cco", bufs=2,
                          addr_space="Shared", name=nm("cco"))
            nc.sync.dma_start(cci, lv)
            nc.gpsimd.collective_compute(
                "AllReduce", ALU.add,
                replica_groups=[list(range(n_cores))],
                ins=[cci.opt()], outs=[cco.opt()])
            gv = svX[b:b + 1, :]
            nc.sync.dma_start(gv, cco)
        else:
            gv = lv
        return lv, gv

    def ac_weights(lv, gv, phase):
        """top-20 mask from gv, softmax of lv/HID over mask -> circulant
        weights written duplicated to DRAM. Returns wd dram tile.
        All vectors at this phase's partition base; X/Y buffers rotate."""
        b = PHASE_BASE[phase]
        S = svS[b:b + 1, :]
        X = svX[b:b + 1, :]
        Y = svY[b:b + 1, :]
        m8, m8b, m8c = S[:, 0:8], S[:, 8:16], S[:, 16:24]
        mx, mxn, sm, rc = (S[:, 24:25], S[:, 25:26], S[:, 26:27], S[:, 27:28])
        nc.vector.max(m8, gv)
        nc.vector.match_replace(Y, in_to_replace=m8, in_values=gv,
                                imm_value=NEG)                    # cur1 = Y
        nc.vector.max(m8b, Y)
        nc.vector.match_replace(X, in_to_replace=m8b, in_values=Y,
                                imm_value=NEG)                    # cur2 = X
        nc.vector.max(m8c, X)
        nc.vector.memset(m8c[:, TOPK - 16:8], NEG)
        nc.vector.match_replace(Y, in_to_replace=m8c, in_values=X,
                                imm_value=NEG)                    # cur3 = Y
        # masked logits: msk*(lv - SMALLNEG) + SMALLNEG
        nc.vector.tensor_scalar(X, Y, float(NEG), None, op0=ALU.is_le)  # msk
        nc.vector.tensor_scalar_add(Y, lv, -float(SMALLNEG))           # ml0
        nc.vector.tensor_mul(X, X, Y)
        nc.vector.tensor_scalar_add(Y, X, float(SMALLNEG))             # ml
        # no max-subtraction: logits are lv/HID with |lv| < ~50 so exp() is
        # always in range, and masked entries (SMALLNEG/HID = -195) still
        # underflow to exactly 0
        nc.scalar.activation(X, Y, AF.Exp, scale=1.0 / HID)            # ex
        nc.vector.reduce_sum(sm, X, axis=mybir.AxisListType.X)
        nc.vector.reciprocal(rc, sm)
        nc.vector.tensor_scalar_mul(Y, X, rc)                          # w
        # circulant source buffer: B[j] = W[(j-1) mod 1024], length 129*1024.
        # R_mt[p, l] = W[(l - (128*mt+p) + 1023) mod 1024] = B[(1024-128*mt)
        # + 1023*p + l] — all-positive strides for the DMA.
        wrot = svh[32 * (phase + 1):32 * (phase + 1) + 1, 0:1024]
        nc.scalar.copy(wrot[:, 0:1], Y[:, 1023:1024])
        nc.scalar.copy(wrot[:, 1:1024], Y[:, 0:1023])
        wd = dp.tile([129 * 1024], F16, tag="wd", bufs=2, name=nm("wd"))
        src = wrot.unsqueeze(1).to_broadcast([1, 129, 1024])
        nc.sync.dma_start(wd, src)
        return wd

    def ac_apply(wd, v, wo_ts, resid_h, a_out_tiles):
        """circulant matmul with v (seq-major tiles), then wo projection with
        residual add. a_out_tiles: list of 4 dest tiles (fp16) for
        wo_out + resid."""
        agg = []
        for cp in range(0, NT, 2):
            pss = [psum_main("circ"), psum_main("circ")]
            for mt in range(8):
                rt = ap_.tile([128, 1024], F16, tag="rt", bufs=2, name=nm("rt"))
                src = bass.AP(wd.tensor, 1024 - 128 * mt,
                              [[1023, 128], [1, 1024]])
                nc.sync.dma_start(rt, src)
                for j, ps in enumerate(pss):
                    cc = cp + j
                    for h in range(2):
                        nc.tensor.matmul(
                            ps[:, h * 512:(h + 1) * 512],
                            lhsT=v[mt][:, cc * 128:(cc + 1) * 128],
                            rhs=rt[:, h * 512:(h + 1) * 512],
                            start=(mt == 0), stop=(mt == 7))
            for j, ps in enumerate(pss):
                ag = ap_.tile([128, 1024], F16, tag=f"th{cp + j}", bufs=1,
                              name=nm("agg"))
                nc.scalar.copy(ag, ps)
                agg.append(ag)

        def evict(cc, ps):
            nc.vector.tensor_add(a_out_tiles[cc], ps, resid_h[cc])
        proj_cm(agg, wo_ts, evict)

    def proj_sm(act_h, wts, tag):
        """v seq-major: out[l, co] = sum_ci act[ci, l-chunk]^T w[ci, co].
        Returns 8 tiles [128, 512] fp16."""
        vts = []
        for lt in range(8):
            ps = pm.tile([128, 512], F32, tag="pmain", bufs=3, name=nm("pv"))
            for ci in range(NT):
                nc.tensor.matmul(ps,
                                 lhsT=act_h[ci][:, lt * 128:(lt + 1) * 128],
                                 rhs=wts[ci],
                                 start=(ci == 0), stop=(ci == NT - 1))
            vt = ap_.tile([128, 512], F16, tag=f"v{lt}", bufs=1, name=nm("v"))
            nc.scalar.copy(vt, ps)
            vts.append(vt)
        return vts

    # ================= conv block =================
    def leaky_evict(dst_ap, ps_ap, n):
        """dst = leaky_relu(ps) = max(ps, 0.01*ps), via fp16 tmp."""
        tmp = ap_.tile([128, 512], F16, tag="lkt", bufs=3, name=nm("lkt"))
        nc.scalar.copy(tmp[:, 0:n], ps_ap)
        nc.vector.scalar_tensor_tensor(
            dst_ap, in0=tmp[:, 0:n], scalar=0.01, in1=tmp[:, 0:n],
            op0=ALU.mult, op1=ALU.max)

    def conv_block(sptiles, w1name, w2name, cf_tag):
        """sptiles: 4 fp16 tiles [128, 1026] (replicate-padded seasonal).
        Returns 4 fp16 tiles [128, 1024]: leaky(conv2(leaky(conv1(s)))) + s."""
        w1, w2 = io[w1name], io[w2name]
        cf = [ap_.tile([128, 1024], F16, tag=f"{cf_tag}{i}", bufs=1,
                       name=nm("cf")) for i in range(NT)]
        for lh in range(2):
            c1h = []
            for co in range(CT):
                w1t = []
                for ci in range(NT):
                    t = wp.tile([128, 384], F16, tag=f"w1_{ci}", bufs=2,
                                name=nm("w1t"))
                    r0 = co * HID + ci * 128
                    nc.sync.dma_start(t, w1[r0:r0 + 128, :])
                    w1t.append(t)
                # 513 true outputs j in [lh*511, lh*511 + 513), computed as
                # 257 cols in psum bank 0 + 256 cols at bank-1 start (col 512)
                # so no matmul crosses a psum bank boundary
                ps = pm.tile([128, 1024], F32, tag="pmain", bufs=3,
                             name=nm("pc1"))
                j0 = 511 * lh
                for ci in range(NT):
                    for tp in range(3):
                        first = (ci == 0 and tp == 0)
                        last = (ci == NT - 1 and tp == 2)
                        for (o0, j1, n) in ((0, 0, 257), (512, 257, 256)):
                            nc.tensor.matmul(
                                ps[:, o0:o0 + n],
                                lhsT=w1t[ci][:, tp * 128:(tp + 1) * 128],
                                rhs=sptiles[ci][:, j0 + j1 + tp:
                                                j0 + j1 + tp + n],
                                start=first, stop=last)
                ch = ap_.tile([128, 514], F16, tag=f"c1_{co}", bufs=1,
                              name=nm("c1h"))
                # tile col t holds output j = lh*511 + t - (1-lh)
                if lh == 0:
                    leaky_evict(ch[:, 1:258], ps[:, 0:257], 257)
                    leaky_evict(ch[:, 258:514], ps[:, 512:768], 256)
                    nc.vector.tensor_copy(ch[:, 0:1], ch[:, 1:2])
                else:
                    leaky_evict(ch[:, 0:257], ps[:, 0:257], 257)
                    leaky_evict(ch[:, 257:513], ps[:, 512:768], 256)
                    nc.vector.tensor_copy(ch[:, 513:514], ch[:, 512:513])
                c1h.append(ch)
            # conv2 for this half: out cols [lh*512, lh*512+512)
            for co in range(NT):
                w2t = []
                for ci in range(CT):
                    t = wp.tile([128, 384], F16, tag=f"w2_{ci}", bufs=1,
                                name=nm("w2t"))
                    r0 = co * CF + ci * 128
                    nc.sync.dma_start(t, w2[r0:r0 + 128, :])
                    w2t.append(t)
                ps = pm.tile([128, 512], F32, tag="pmain", bufs=3,
                             name=nm("pc2"))
                # c2 out col l = lh*512 + t needs c1[l + tp - 1], which lives
                # at c1h col t + tp for both halves
                for ci in range(CT):
                    for tp in range(3):
                        nc.tensor.matmul(
                            ps,
                            lhsT=w2t[ci][:, tp * 128:(tp + 1) * 128],
                            rhs=c1h[ci][:, tp:tp + 512],
                            start=(ci == 0 and tp == 0),
                            stop=(ci == CT - 1 and tp == 2))
                tmp = ap_.tile([128, 512], F16, tag="c2t", bufs=2,
                               name=nm("c2t"))
                leaky_evict(tmp, ps, 512)
                nc.vector.tensor_add(
                    cf[co][:, lh * 512:(lh + 1) * 512], tmp,
                    sptiles[co][:, lh * 512 + 1:lh * 512 + 513])
        return cf

    # ================= layer norm =================
    def layer_norm(x_h, g_ap, b_ap, out_tiles, lnphase):
        """x_h: 4 fp16 tiles [128,1024] channel-major. out: 4 fp16 tiles."""
        psx = psum_vec("lnx")
        psq = psum_vec("lnq")
        for ci in range(NT):
            sq = ap_.tile([128, 1024], F16, tag="sqh", bufs=2, name=nm("sq"))
            nc.scalar.square(sq, x_h[ci])
            for h in range(2):
                nc.tensor.matmul(psx[:, h * 512:(h + 1) * 512],
                                 lhsT=ones[:, 0:1],
                                 rhs=x_h[ci][:, h * 512:(h + 1) * 512],
                                 start=(ci == 0), stop=(ci == NT - 1))
                nc.tensor.matmul(psq[:, h * 512:(h + 1) * 512],
                                 lhsT=ones[:, 0:1],
                                 rhs=sq[:, h * 512:(h + 1) * 512],
                                 start=(ci == 0), stop=(ci == NT - 1))
        mean = svL[96:97, :]
        nc.vector.tensor_scalar_mul(mean, psx, 1.0 / HID)
        var = svX[96:97, :]
        # var = psq/HID - mean^2
        nc.vector.tensor_mul(var, mean, mean)
        nc.vector.scalar_tensor_tensor(var, in0=psq, scalar=1.0 / HID,
                                       in1=var, op0=ALU.mult, op1=ALU.subtract)
        std = svY[96:97, :]
        nc.scalar.activation(std, var, AF.Sqrt, bias=epsct[96:97, :])
        rstd = svX[96:97, :]   # var dead
        nc.vector.reciprocal(rstd, std)
        mh = svh[0:1, 0:1024]
        nc.vector.tensor_copy(mh, mean)
        rh = svh[0:1, 1024:2048]
        nc.vector.tensor_copy(rh, rstd)
        # broadcast via rank-1 matmul
        psb_m = psum_main("bcm")
        psb_r = psum_main("bcr")
        for h in range(2):
            nc.tensor.matmul(psb_m[:, h * 512:(h + 1) * 512],
                             lhsT=ones[0:1, 0:128],
                             rhs=mh[:, h * 512:(h + 1) * 512],
                             start=True, stop=True)
            nc.tensor.matmul(psb_r[:, h * 512:(h + 1) * 512],
                             lhsT=ones[0:1, 0:128],
                             rhs=rh[:, h * 512:(h + 1) * 512],
                             start=True, stop=True)
        for ci in range(NT):
            t1 = ap_.tile([128, 1024], F16, tag="lnt", bufs=2, name=nm("lnt"))
            nc.vector.tensor_sub(t1, x_h[ci], psb_m)
            nc.vector.tensor_mul(t1, t1, psb_r)
            nc.vector.tensor_scalar(out_tiles[ci], t1,
                                    g_ap[:, ci:ci + 1], b_ap[:, ci:ci + 1],
                                    op0=ALU.mult, op1=ALU.add)

    # ======================================================================
    # phase A — input prep + embeddings
    # ======================================================================
    with nc.named_scope("prep"):
        xtd = fp.tile([INF, 1024], F32, tag="cs", bufs=1, name="xtd")
        nc.sync.dma_start(xtd, io["xt"])
        # DVE bounce: gives scan/stt consumers a same-engine producer
        xtf = fp.tile([INF, 1024], F32, tag="af", bufs=2, name="xtf")
        nc.vector.tensor_copy(xtf, xtd)
        xth = ap_.tile([INF, 1024], F16, tag="lnt", bufs=2, name="xth")
        nc.scalar.copy(xth, xtf)

    # initial decomposition: need the window-sum tensor D directly (both
    # seasonal and trend halves are consumed), so inline it here
    with nc.named_scope("initdecomp"):
        dx = fp.tile([INF, 1024], F32, tag="dwin", bufs=1, name="dx")
        cs0 = fp.tile([128, 1024], F32, tag="cs", bufs=1, name="cs0")
        cs0 = cs0[:INF, :]
        nc.vector.tensor_tensor_scan(
            cs0, xtf, zc[:INF, :].to_broadcast([INF, 1024]),
            0.0, ALU.add, ALU.add)
        d0 = dx
        nc.vector.tensor_sub(d0[:, 13:1012], cs0[:, 25:1024], cs0[:, 0:999])
        nc.vector.tensor_copy(d0[:, 0:13], cs0[:, 12:25])
        nc.vector.scalar_tensor_tensor(
            d0[:, 1012:1024], in0=cs0[:, 999:1011], scalar=-1.0,
            in1=cs0[:, 1023:1024].to_broadcast([INF, 12]),
            op0=ALU.mult, op1=ALU.add)
        nc.vector.scalar_tensor_tensor(
            d0[:, 0:12], in0=coefL[:INF, :], scalar=xtf[:, 0:1],
            in1=d0[:, 0:12], op0=ALU.mult, op1=ALU.add)
        nc.vector.scalar_tensor_tensor(
            d0[:, 1012:1024], in0=coefR[:INF, :], scalar=xtf[:, 1023:1024],
            in1=d0[:, 1012:1024], op0=ALU.mult, op1=ALU.add)

        # seasonal_init: cols 0:512 = (x - D/25)[:, 512:], cols 512: = 0
        sih = ap_.tile([INF, 1024], F16, tag="sqh", bufs=2, name="sih")
        nc.vector.scalar_tensor_tensor(
            sih[:, 0:512], in0=d0[:, 512:1024], scalar=-1.0 / KD,
            in1=xtf[:, 512:1024], op0=ALU.mult, op1=ALU.add)
        nc.vector.memset(sih[:, 512:1024], 0.0)
        # trend_init: cols 0:512 = D[:, 512:]/25, cols 512: = mean(x)
        tih = ap_.tile([INF, 1024], F16, tag="sqh", bufs=2, name="tih")
        nc.vector.tensor_scalar_mul(tih[:, 0:512], d0[:, 512:1024], 1.0 / KD)
        mnx = sp.tile([INF, 1], F32, tag="mnx", bufs=1, name="mnx")
        nc.vector.reduce_sum(mnx, xtf, axis=mybir.AxisListType.X)
        nc.vector.tensor_scalar_mul(mnx, mnx, 1.0 / L)
        nc.vector.tensor_scalar(
            tih[:, 512:1024],
            zc[:INF, :].to_broadcast([INF, 512]), mnx, None,
            op0=ALU.add)

    with nc.named_scope("embed"):
        wemb = [wp.tile([INF, HID], F16, tag="pw", bufs=8, name="wemb")]
        nc.sync.dma_start(wemb[0], io["we"])

        def emb(src_h, dst_tag, f32_dst=False, bufs=1):
            outs = []
            for cc in range(NT):
                ps = psum_main("emb")
                for h in range(2):
                    nc.tensor.matmul(
                        ps[:, h * 512:(h + 1) * 512],
                        lhsT=wemb[0][:, cc * 128:(cc + 1) * 128],
                        rhs=src_h[:, h * 512:(h + 1) * 512],
                        start=True, stop=True)
                if f32_dst:
                    o = fp.tile([128, 1024], F32, tag=f"{dst_tag}{cc}",
                                bufs=bufs, name=nm(dst_tag))
                    nc.vector.tensor_scalar_add(o, ps, bemb[:, cc:cc + 1])
                else:
                    o = ap_.tile([128, 1024], F16, tag=f"{dst_tag}{cc}",
                                 bufs=bufs, name=nm(dst_tag))
                    nc.scalar.activation(o, ps, AF.Identity,
                                         bias=bemb[:, cc:cc + 1])
                outs.append(o)
            return outs

        xe = emb(xth, "xe")
        se = emb(sih, "se")
        trend = emb(tih, "tr", f32_dst=True)  # running trend accumulator

    # ======================================================================
    # encoder autocorrelation (+ dec-ac1 gram interleaved to hide latency)
    # ======================================================================
    with nc.named_scope("enc_ac_gram"):
        wq = load_proj_w("e_wq")
        wk = load_proj_w("e_wk")
        q = []
        k = []
        for cc in range(NT):
            qt = ap_.tile([128, 1024], F16, tag=f"q{cc}", bufs=1, name=nm("q"))
            kt = ap_.tile([128, 1024], F16, tag=f"k{cc}", bufs=1, name=nm("k"))
            q.append(qt)
            k.append(kt)
        proj_cm(xe, wq, lambda cc, ps: nc.scalar.copy(q[cc], ps))
        proj_cm(xe, wk, lambda cc, ps: nc.scalar.copy(k[cc], ps))
        lv_e, gv_e = ac_gram(q, k, 0)

    with nc.named_scope("dec1_ac_gram"):
        wq1 = load_proj_w("d1_wq")
        wk1 = load_proj_w("d1_wk")
        q1 = []
        k1 = []
        for cc in range(NT):
            qt = ap_.tile([128, 1024], F16, tag=f"q{cc}", bufs=1, name=nm("q1"))
            kt = ap_.tile([128, 1024], F16, tag=f"k{cc}", bufs=1, name=nm("k1"))
            q1.append(qt)
            k1.append(kt)
        proj_cm(se, wq1, lambda cc, ps: nc.scalar.copy(q1[cc], ps))
        proj_cm(se, wk1, lambda cc, ps: nc.scalar.copy(k1[cc], ps))
        lv_1, gv_1 = ac_gram(q1, k1, 1)

    with nc.named_scope("enc_ac_apply"):
        wv_ = load_proj_w("e_wv")
        v = proj_sm(xe, wv_, "v")
        wd_e = ac_weights(lv_e, gv_e, 0)
        wo_ = load_proj_w("e_wo")
        a_enc = [fp.tile([128, 1024], F32, tag="af", bufs=2, name=nm("ae"))
                 for _ in range(NT)]
        ac_apply(wd_e, v, wo_, xe, a_enc)

    # ======================================================================
    # encoder decomp 1 -> s1 (padded), convs, decomp 2, LN
    # ======================================================================
    with nc.named_scope("enc_decomp1"):
        s1p = [ap_.tile([128, 1026], F16, tag=f"sp{i}", bufs=1, name=nm("s1p"))
               for i in range(NT)]
        for cc in range(NT):
            decomp(a_enc[cc], s1p[cc][:, 1:1025])
            nc.vector.tensor_copy(s1p[cc][:, 0:1], s1p[cc][:, 1:2])
            nc.vector.tensor_copy(s1p[cc][:, 1025:1026], s1p[cc][:, 1024:1025])

    with nc.named_scope("enc_convs"):
        cf_e = conv_block(s1p, "e_w1", "e_w2", "cf")

    with nc.named_scope("enc_ln"):
        sf = [ap_.tile([128, 1024], F16, tag=f"sf{i}", bufs=1, name=nm("sf"))
              for i in range(NT)]
        for cc in range(NT):
            decomp(cf_e[cc], sf[cc])
        enc_out = [ap_.tile([128, 1024], F16, tag=f"eo{i}", bufs=1,
                            name=nm("eo")) for i in range(NT)]
        layer_norm(sf, lnw["elng"], lnw["elnb"], enc_out, 0)

    # ======================================================================
    # decoder ac1 apply -> a1 -> decomp -> s1d, t1 -> lin1 -> trend
    # ======================================================================
    with nc.named_scope("dec1_apply"):
        wv1 = load_proj_w("d1_wv")
        v1 = proj_sm(se, wv1, "v")
        wd_1 = ac_weights(lv_1, gv_1, 1)
        wo1 = load_proj_w("d1_wo")
        a1 = [fp.tile([128, 1024], F32, tag="af", bufs=2, name=nm("a1"))
              for _ in range(NT)]
        ac_apply(wd_1, v1, wo1, se, a1)

    with nc.named_scope("dec1_decomp"):
        s1d = [ap_.tile([128, 1024], F16, tag=f"s1d{i}", bufs=1, name=nm("s1d"))
               for i in range(NT)]
        th = [ap_.tile([128, 1024], F16, tag=f"th{i}", bufs=1, name=nm("th"))
              for i in range(NT)]
        for cc in range(NT):
            decomp(a1[cc], s1d[cc], t_out=th[cc])
        wl1 = load_proj_w("lin1")

        def ev_t1(cc, ps):
            nc.vector.tensor_add(trend[cc], trend[cc], ps)
        proj_cm(th, wl1, ev_t1)

    # ======================================================================
    # decoder ac2: q from s1d, k/v from enc_out
    # ======================================================================
    with nc.named_scope("dec2_ac"):
        wq2 = load_proj_w("d2_wq")
        wk2 = load_proj_w("d2_wk")
        q2 = []
        k2 = []
        for cc in range(NT):
            qt = ap_.tile([128, 1024], F16, tag=f"q{cc}", bufs=1, name=nm("q2"))
            kt = ap_.tile([128, 1024], F16, tag=f"k{cc}", bufs=1, name=nm("k2"))
            q2.append(qt)
            k2.append(kt)
        proj_cm(s1d, wq2, lambda cc, ps: nc.scalar.copy(q2[cc], ps))
        proj_cm(enc_out, wk2, lambda cc, ps: nc.scalar.copy(k2[cc], ps))
        lv_2, gv_2 = ac_gram(q2, k2, 2)
        wv2 = load_proj_w("d2_wv")
        v2 = proj_sm(enc_out, wv2, "v")
        wd_2 = ac_weights(lv_2, gv_2, 2)
        wo2 = load_proj_w("d2_wo")
        a2 = [fp.tile([128, 1024], F32, tag="af", bufs=2, name=nm("a2"))
              for _ in range(NT)]
        ac_apply(wd_2, v2, wo2, s1d, a2)

    with nc.named_scope("dec2_decomp"):
        s2p = [ap_.tile([128, 1026], F16, tag=f"sp{i}", bufs=1, name=nm("s2p"))
               for i in range(NT)]
        th2 = [ap_.tile([128, 1024], F16, tag=f"th{i}", bufs=1, name=nm("th2"))
               for i in range(NT)]
        for cc in range(NT):
            decomp(a2[cc], s2p[cc][:, 1:1025], t_out=th2[cc])
            nc.vector.tensor_copy(s2p[cc][:, 0:1], s2p[cc][:, 1:2])
            nc.vector.tensor_copy(s2p[cc][:, 1025:1026], s2p[cc][:, 1024:1025])
        wl2 = load_proj_w("lin2")

        def ev_t2(cc, ps):
            nc.vector.tensor_add(trend[cc], trend[cc], ps)
        proj_cm(th2, wl2, ev_t2)

    # ======================================================================
    # decoder convs, decomp 3, LN, season + trend, final projection
    # ======================================================================
    with nc.named_scope("dec_convs"):
        cf_d = conv_block(s2p, "d_w1", "d_w2", "cf")

    with nc.named_scope("dec_final"):
        s3 = [ap_.tile([128, 1024], F16, tag=f"sf{i}", bufs=1, name=nm("s3"))
              for i in range(NT)]
        th3 = [ap_.tile([128, 1024], F16, tag=f"th{i}", bufs=1, name=nm("th3"))
               for i in range(NT)]
        for cc in range(NT):
            decomp(cf_d[cc], s3[cc], t_out=th3[cc])
        wl3 = load_proj_w("lin3")

        def ev_t3(cc, ps):
            nc.vector.tensor_add(trend[cc], trend[cc], ps)
        proj_cm(th3, wl3, ev_t3)

        sea = [ap_.tile([128, 1024], F16, tag=f"xe{i}", bufs=1, name=nm("sea"))
               for i in range(NT)]
        layer_norm(s3, lnw["dlng"], lnw["dlnb"], sea, 1)
        wls = load_proj_w("lins")
        fin = [ap_.tile([128, 1024], F16, tag=f"q{i}", bufs=1,
                        name=nm("fin")) for i in range(NT)]

        def ev_sea(cc, ps):
            nc.vector.tensor_add(fin[cc], ps, trend[cc])
        proj_cm(sea, wls, ev_sea)

        # final: out[l, co] for l in [512, 1024)
        wfin = [wp.tile([128, INF], F16, tag="pw", bufs=8, name=nm("wfin"))
                for _ in range(NT)]
        for ci in range(NT):
            nc.sync.dma_start(wfin[ci], io["wf"][ci * 128:(ci + 1) * 128, :])
        for lt in range(4, 8):
            ps = pm.tile([128, INF], F32, tag="pmain", bufs=3, name=nm("pf"))
            for ci in range(NT):
                nc.tensor.matmul(ps,
                                 lhsT=fin[ci][:, lt * 128:(lt + 1) * 128],
                                 rhs=wfin[ci],
                                 start=(ci == 0), stop=(ci == NT - 1))
            of = sp.tile([128, INF], F32, tag="of", bufs=2, name=nm("of"))
            nc.vector.tensor_add(of, ps, bfin)
            nc.sync.dma_start(
                out[(lt - 4) * 128:(lt - 3) * 128, :], of)

    ctx.close()


# --------------------------------------------------------------------------
# host driver
# --------------------------------------------------------------------------

def _prep_inputs(x, params):
    """Returns list of per-core in_maps."""
    g = lambda *ks: np.asarray(_dig(params, ks))
    shared = {}
    shared["we"] = g("w_emb").astype(np.float16)
    shared["wf"] = g("w_final").astype(np.float16)
    shared["bemb"] = np.ascontiguousarray(
        g("b_emb").reshape(NT, 128).T.astype(np.float32))
    bf = g("b_final").astype(np.float32)
    shared["bfin"] = np.ascontiguousarray(
        np.broadcast_to(bf[None, :], (128, INF)).copy())
    shared["coefL"] = np.ascontiguousarray(np.broadcast_to(
        np.arange(12, 0, -1, dtype=np.float32)[None, :], (128, 12)).copy())
    shared["coefR"] = np.ascontiguousarray(np.broadcast_to(
        np.arange(1, 13, dtype=np.float32)[None, :], (128, 12)).copy())
    shared["onesh"] = np.ones((128, 128), np.float16)
    acmap = {"e": ("enc", "ac"), "d1": ("dec", "ac1"), "d2": ("dec", "ac2")}
    for pfx, ks in acmap.items():
        for wn in ("wq", "wk", "wv", "wo"):
            shared[f"{pfx}_{wn}"] = g(*ks, wn).astype(np.float16)
    for i, wn in enumerate(("lin1", "lin2", "lin3")):
        shared[wn] = g("dec", wn).astype(np.float16)
    shared["lins"] = g("dec", "lin_season").astype(np.float16)
    for pfx, side in (("e", "enc"), ("d", "dec")):
        w1 = g(side, "conv1")  # [CF, HID, 3]
        w1r = w1.reshape(CT, 128, HID, 3).transpose(0, 2, 3, 1)
        shared[f"{pfx}_w1"] = np.ascontiguousarray(
            w1r.reshape(CT * HID, 384)).astype(np.float16)
        w2 = g(side, "conv2")  # [HID, CF, 3]
        w2r = w2.reshape(NT, 128, CF, 3).transpose(0, 2, 3, 1)
        shared[f"{pfx}_w2"] = np.ascontiguousarray(
            w2r.reshape(NT * CF, 384)).astype(np.float16)
        shared[f"{pfx}_lng"] = np.ascontiguousarray(
            g(side, "ln_g").reshape(NT, 128).T.astype(np.float32))
        shared[f"{pfx}_lnb"] = np.ascontiguousarray(
            g(side, "ln_b").reshape(NT, 128).T.astype(np.float32))

    in_maps = []
    for c in range(N_CORES):
        m = dict(shared)
        m["xt"] = np.ascontiguousarray(np.asarray(x[c]).T.astype(np.float32))
        in_maps.append(m)
    return in_maps


def _dig(d, ks):
    for k in ks:
        d = d[k]
    return d


def kernel(x, params):
    global LAST_RESULT, _CACHED
    import os
    try:
        import antenv.axon_hooks  # noqa: F401
    except ImportError:
        # tracing under axon needs this hook; without it a stray BASS_TRACE
        # in the environment would crash the run
        os.environ["BASS_NEVER_TRACE"] = "1"
    x = np.asarray(x)
    if _CACHED is None:
        _CACHED = build_program(N_CORES)
    nc = _CACHED
    in_maps = _prep_inputs(x, params)
    res = run_bass_kernel_spmd(nc, in_maps, core_ids=list(range(N_CORES)))
    LAST_RESULT = res
    outs = np.stack([r["out"] for r in res.results], axis=0)
    return outs.astype(np.float32)


if __name__ == "__main__":
    nc = build_program(1)
    print("program built OK")
